# revision 22
# baseline (speedup 1.0000x reference)
"""DMV inside algorithm (Eisner chart DP, logsumexp semiring) on Trainium2.

Strategy
--------
Pure data parallelism over the batch: 4096 sentences -> 8 cores x 512.
Per core: ONE run of 512 sentences laid out as [128 SBUF partitions] x
[G=4 sentence groups in the free dim], all chart tables in bf16.

The DP runs in the *exp domain* (no per-split transcendentals): tables hold
exp(score). Each width-w update is one fused strided multiply (products
P[qg,t,i]) followed by an in-place folding tree of adds that reduces over
the split dim t. Everything iterates [qg, t, i] with i innermost and
stride 1, which (with bf16) hits the DVE 2x packed-16-bit mode; the tree
of tensor_tensor adds also runs at 2x, unlike InstTensorReduce which gets
no fast mode.

Tables are stored *diagonal-packed*: Xd[d*41 + i] = X[i, i+d], making
every gather in the width-w recurrence a constant-stride access pattern.
IR/IL are stored with row r holding width r+1 (IL column-shifted by +1)
so all four quantities' gathers share one AP shape.

Numerics: scale composes linearly in span width (every width-w entry has
exactly w arcs), so one on-device renormalization at w=20 multiplies row
d by an exact power of two 2^(-k*d) (k integer per sentence), keeping
everything in range (bf16 range == f32 range). k is returned per sentence
and undone on the host: LL = log(CR[0,len]) + k*ln2*len + c0*len.
"""

import os

os.environ.setdefault("JAX_PLATFORMS", "cpu")

import numpy as np
import ml_dtypes

import concourse.bass as bass  # noqa: F401  (registers engine classes)
import concourse.tile as tile
import bass_rust
from concourse import bacc, mybir

F32 = mybir.dt.float32
BF16 = mybir.dt.bfloat16
AF = mybir.ActivationFunctionType
OP = mybir.AluOpType
AX = mybir.AxisListType

N = 41              # fake_len (ROOT at 0)
D = 1681            # table pitch: N*N elements
G = 4               # sentence groups per partition
NCORES = 8
B_CORE = 128 * G    # 512
CONST_IN = 4 * D    # host sends 4 pre-exponentiated tables/sentence (bf16)
STOP_IN = 4 * N     # host sends 4 exp'd stop vectors/sentence (bf16)
RENORM_W = 20

# banks tile (bf16): 24 diag-packed tables, slot k at offset k*D.
# Since every DP multiply is a per-direction 4-slot op, no table needs a
# second copy; all paired-access strides stay within 16*D (ISA AP steps
# are 16-bit).
#   KL g0..g3: 0..3    KR: 4..7    CRa: 8..11   CLa: 12..15
#   IR: 16..19         IL: 20..23  (IL stored col+1)
S_KL, S_KR, S_CRA, S_CLA, S_IR, S_IL = 0, 4, 8, 12, 16, 20
# consts tile (bf16): 4 per-arc tables, g-major: offset (4*g + C)*D
# (A0/B0 kept verbatim, not as differences: all-positive arithmetic so
# bf16 never hits catastrophic cancellation)
C_A1, C_B1, C_A0, C_B0 = range(4)
# stops tile (bf16): 16 vectors of 41, offset (g*4 + v)*41
V_SLNO, V_SLHAS, V_SRNO, V_SRHAS = range(4)

# bf16 scratch tile element offsets
ZB_P = 0          # 3360: products [qg, t, i] (qg-stride = s*w, varies by w)
ZB_T1 = 3360      # 328: tmp1 [q, g, i]
ZB_T2 = 3688      # 328
ZB_SS = 4016      # 328: reduce-path sums [q, g, i]
ZB_MX = 4344      # 4*22*41: expanded renorm multiplier Mx[g, d, i] = 2^(-k_g*d)
ZB_TOTAL = 7952

# f32 scratch tile element offsets
ZF_M2 = 0         # 8
ZF_MU = 8         # 4
ZF_LM = 12        # 4 (reused for k)
ZF_M = 16         # 4*42: renorm multiplier table [g, 42]
ZF_CROUT = 184    # 4*41
ZF_DSUM = 348     # 4
ZF_TOTAL = 352

LN2_32 = 32.0 * float(np.log(2.0))

# from this width on, the g3 lane of opA-L / opB-R (mult + folds) runs on
# the otherwise-idle Pool engine; below it Pool's per-op launch overhead
# would make the Pool chain longer than the whole DVE op
POOL_W0 = 7


def ap_of(t, offset, dims, lead=None):
    """Build a raw AP on tile/dram ap `t`: [lead or t.ap[0]] + dims."""
    ap = t.copy()
    first = list(t.ap[0]) if lead is None else list(lead)
    ap.ap = bass_rust.VecI64Pair([first] + [list(d) for d in dims])
    ap.offset = offset
    return ap


def build_nc():
    nc = bacc.Bacc("TRN2", target_bir_lowering=False, debug=False, num_devices=1)
    consts_in = nc.dram_tensor("consts", [B_CORE, CONST_IN], BF16, kind="ExternalInput").ap()
    stops_in = nc.dram_tensor("stops", [B_CORE, STOP_IN], BF16, kind="ExternalInput").ap()
    logs_d = nc.dram_tensor("ecr", [B_CORE, N], F32, kind="ExternalOutput").ap()
    dsum_d = nc.dram_tensor("dsum", [B_CORE], F32, kind="ExternalOutput").ap()

    with tile.TileContext(nc) as tc:
        with tc.tile_pool(name="p", bufs=1) as pool, \
                nc.allow_low_precision(reason="bf16 chart DP by design"):
            banks_t = pool.tile([128, 24 * D], BF16)
            consts_t = pool.tile([128, 16 * D], BF16)
            stops_t = pool.tile([128, 16 * N], BF16)
            zb_t = pool.tile([128, ZB_TOTAL], BF16)
            zf_t = pool.tile([128, ZF_TOTAL], F32)
            banks = banks_t[:]
            consts = consts_t[:]
            stops = stops_t[:]
            zb = zb_t[:]
            zf = zf_t[:]

            v = nc.vector
            sc = nc.scalar

            # ---- load host-precomputed exp-domain constants ----
            nc.sync.dma_start(
                ap_of(stops, 0, [[STOP_IN, G], [1, STOP_IN]]),
                ap_of(stops_in, 0,
                      [[STOP_IN, G], [1, STOP_IN]], lead=[G * STOP_IN, 128]),
            )
            # consts split by row range: step w reads row w only, so later
            # chunks' DMA hides under early DP steps
            # chunks of >=7 rows: a chunk of one table is (hi-lo)*82B
            # contiguous in DRAM, and >=512B descriptors avoid the small-
            # descriptor DMA penalty
            for lo, hi in ((0, 7 * N), (7 * N, 14 * N), (14 * N, 22 * N),
                           (22 * N, 31 * N), (31 * N, D)):
                nc.sync.dma_start(
                    ap_of(consts, lo, [[4 * D, G], [D, 4], [1, hi - lo]]),
                    ap_of(consts_in, lo, [[CONST_IN, G], [D, 4], [1, hi - lo]],
                          lead=[G * CONST_IN, 128]),
                )

            # ---- width-0 init ----
            # The renorm rescale reads full N-wide rows; the tail columns the
            # DP never writes hold whatever SBUF held before. That is safe:
            # those cells are only ever read by the renorm itself, which
            # multiplies them in place — garbage never reaches a DP-read cell,
            # so no zero-init is needed at all.
            v.memset(ap_of(zf, ZF_DSUM, [[1, G]]), 0.0)
            # KR[0,:] = KL[0,:] = 1
            v.memset(ap_of(banks, S_KL * D, [[4 * D, 2], [D, 4], [1, N]]), 1.0)
            # CRa[0,i] = exp(stop[i,R,NO]); CLa[0,i] = exp(stop[i,L,NO])
            v.tensor_copy(
                ap_of(banks, S_CRA * D, [[D, 4], [1, N]]),
                ap_of(stops, V_SRNO * N, [[4 * N, 4], [1, N]]),
            )
            v.tensor_copy(
                ap_of(banks, S_CLA * D, [[D, 4], [1, N]]),
                ap_of(stops, V_SLNO * N, [[4 * N, 4], [1, N]]),
            )

            def fold_chains(w, s, t0, count, chains):
                """In-place fold slot-chains of P[qg, t, i] over t in
                [t0, t0+count) down to one row at t0, interleaving the
                independent chains so they hide each other's semaphore
                latency. Each chain is (engine, first_slot, n_slots, final):
                `final` (if given) redirects that chain's last fold."""
                sw = s * w
                h = count
                while h > 1:
                    h2 = h // 2
                    hc = h - h2
                    for eng, slot0, nsl, fin in chains:
                        base = ZB_P + slot0 * sw + t0 * s
                        if hc == 1 and fin is not None:
                            out = fin
                        else:
                            out = ap_of(zb, base, [[sw, nsl], [s, h2], [1, s]])
                        eng.tensor_tensor(
                            out,
                            ap_of(zb, base, [[sw, nsl], [s, h2], [1, s]]),
                            ap_of(zb, base + hc * s, [[sw, nsl], [s, h2], [1, s]]),
                            OP.add,
                        )
                    h = hc

            # ---- chart DP ----
            for w in range(1, N):
                s = N - w
                sw = s * w
                row = (w - 1) * N + 1
                # opA: NOCHILD edges (t=0 for q=R, t=w-1 for q=L) have a
                # trivial K-factor of 1, so products cover only the w-1
                # HASCHILD splits; the edges are read straight from banks.
                # tmp2[q,g,i] = NOCHILD edge * {A0,B0}[w,:]
                v.tensor_tensor(
                    ap_of(zb, ZB_T2, [[4 * s, 2], [s, 4], [1, s]]),
                    ap_of(banks, S_CLA * D + (w - 1) * N + 1,
                          [[-4 * D - 1, 2], [D, 4], [1, s]]),
                    ap_of(consts, C_A0 * D + w * N, [[D, 2], [4 * D, 4], [1, s]]),
                    OP.mult,
                )
                if w > 1:
                    sw1 = s * (w - 1)
                    pool_w = POOL_W0 <= w
                    # q=R forward, t in [1,w): P[g,t,i] = KR[t,i]*CLa[w-1-t, i+t+1]
                    v.tensor_tensor(
                        ap_of(zb, ZB_P, [[sw1, 4], [s, w - 1], [1, s]]),
                        ap_of(banks, S_KR * D + N, [[D, 4], [N, w - 1], [1, s]]),
                        ap_of(banks, S_CLA * D + row - 40, [[D, 4], [-40, w - 1], [1, s]]),
                        OP.mult,
                    )
                    # q=L t-reversed, t' in [1,w): P[4+g,t',i] = CRa[w-1-t',i]*KL[t', i+w-t']
                    # (g3 of this half runs on the otherwise-idle Pool engine)
                    ng = 3 if pool_w else 4
                    v.tensor_tensor(
                        ap_of(zb, ZB_P + 4 * sw1, [[sw1, ng], [s, w - 1], [1, s]]),
                        ap_of(banks, S_CRA * D + (w - 2) * N, [[D, ng], [-N, w - 1], [1, s]]),
                        ap_of(banks, S_KL * D + N + w - 1, [[D, ng], [40, w - 1], [1, s]]),
                        OP.mult,
                    )
                    if pool_w:
                        nc.gpsimd.tensor_tensor(
                            ap_of(zb, ZB_P + 7 * sw1, [[sw1, 1], [s, w - 1], [1, s]]),
                            ap_of(banks, (S_CRA + 3) * D + (w - 2) * N, [[D, 1], [-N, w - 1], [1, s]]),
                            ap_of(banks, (S_KL + 3) * D + N + w - 1, [[D, 1], [40, w - 1], [1, s]]),
                            OP.mult,
                        )
                        fold_chains(w - 1, s, 0, w - 1, [
                            (v, 0, 4, None), (v, 4, 3, None), (nc.gpsimd, 7, 1, None)])
                    else:
                        fold_chains(w - 1, s, 0, w - 1, [(v, 0, 4, None), (v, 4, 4, None)])
                    # tmp1[q,g,i] = (sum over HASCHILD splits) * {A1,B1}[w,:]
                    v.tensor_tensor(
                        ap_of(zb, ZB_T1, [[4 * s, 2], [s, 4], [1, s]]),
                        ap_of(zb, ZB_P, [[4 * sw1, 2], [sw1, 4], [1, s]]),
                        ap_of(consts, C_A1 * D + w * N, [[D, 2], [4 * D, 4], [1, s]]),
                        OP.mult,
                    )
                    # IR[w-1, i] = tmp1R + tmp2R; IL[w-1, i+1] = tmp1L + tmp2L
                    # (two ops so each opB mult waits only on its own input)
                    v.tensor_tensor(
                        ap_of(banks, S_IR * D + (w - 1) * N, [[D, 4], [1, s]]),
                        ap_of(zb, ZB_T1, [[s, 4], [1, s]]),
                        ap_of(zb, ZB_T2, [[s, 4], [1, s]]),
                        OP.add,
                    )
                    v.tensor_tensor(
                        ap_of(banks, S_IL * D + (w - 1) * N + 1, [[D, 4], [1, s]]),
                        ap_of(zb, ZB_T1 + 4 * s, [[s, 4], [1, s]]),
                        ap_of(zb, ZB_T2 + 4 * s, [[s, 4], [1, s]]),
                        OP.add,
                    )
                else:
                    # w=1: only the NOCHILD edge exists: IR/IL row 0 = tmp2
                    v.tensor_copy(
                        ap_of(banks, S_IR * D, [[4 * D + 1, 2], [D, 4], [1, s]]),
                        ap_of(zb, ZB_T2, [[4 * s, 2], [s, 4], [1, s]]),
                    )
                # opB products, half 0 (q=L): P[g,t,i] = CLa[t,i]*IL[w-1-t, i+t+1]
                # half 1 (q=R): P[4+g,t,i] = IR[t,i]*CRa[w-1-t, i+t+1]
                klout = ap_of(banks, S_KL * D + w * N, [[D, 4], [1, s]])
                krout = ap_of(banks, S_KR * D + w * N, [[D, 4], [1, s]])
                if w == 1:
                    v.tensor_tensor(
                        klout,
                        ap_of(banks, S_CLA * D, [[D, 4], [N, 1], [1, s]]),
                        ap_of(banks, S_IL * D + row, [[D, 4], [-40, 1], [1, s]]),
                        OP.mult,
                    )
                    v.tensor_tensor(
                        krout,
                        ap_of(banks, S_IR * D, [[D, 4], [N, 1], [1, s]]),
                        ap_of(banks, S_CRA * D + row, [[D, 4], [-40, 1], [1, s]]),
                        OP.mult,
                    )
                else:
                    pool_w = POOL_W0 <= w
                    v.tensor_tensor(
                        ap_of(zb, ZB_P, [[sw, 4], [s, w], [1, s]]),
                        ap_of(banks, S_CLA * D, [[D, 4], [N, w], [1, s]]),
                        ap_of(banks, S_IL * D + row, [[D, 4], [-40, w], [1, s]]),
                        OP.mult,
                    )
                    ng = 3 if pool_w else 4
                    v.tensor_tensor(
                        ap_of(zb, ZB_P + 4 * sw, [[sw, ng], [s, w], [1, s]]),
                        ap_of(banks, S_IR * D, [[D, ng], [N, w], [1, s]]),
                        ap_of(banks, S_CRA * D + row, [[D, ng], [-40, w], [1, s]]),
                        OP.mult,
                    )
                    if pool_w:
                        nc.gpsimd.tensor_tensor(
                            ap_of(zb, ZB_P + 7 * sw, [[sw, 1], [s, w], [1, s]]),
                            ap_of(banks, (S_IR + 3) * D, [[D, 1], [N, w], [1, s]]),
                            ap_of(banks, (S_CRA + 3) * D + row, [[D, 1], [-40, w], [1, s]]),
                            OP.mult,
                        )
                        kr3 = ap_of(banks, (S_KR + 3) * D + w * N, [[D, 1], [1, s]])
                        krout3 = ap_of(banks, S_KR * D + w * N, [[D, 3], [1, s]])
                        fold_chains(w, s, 0, w, [
                            (v, 0, 4, klout), (v, 4, 3, krout3), (nc.gpsimd, 7, 1, kr3)])
                    else:
                        fold_chains(w, s, 0, w, [(v, 0, 4, klout), (v, 4, 4, krout)])
                # CRa[w,i] = KR[w,i]*sRhas[i]; CLa[w,i] = KL[w,i]*sLhas[i+w]
                v.tensor_tensor(
                    ap_of(banks, S_CRA * D + w * N, [[4 * D, 2], [D, 4], [1, s]]),
                    ap_of(banks, S_KR * D + w * N, [[-4 * D, 2], [D, 4], [1, s]]),
                    ap_of(stops, V_SRHAS * N, [[w - 2 * N, 2], [4 * N, 4], [1, s]]),
                    OP.mult,
                )

                if w == RENORM_W:
                    s0 = N - w
                    # mu[g] = max_i max(KR[w,i], KL[w,i])  (per partition)
                    v.tensor_reduce(
                        ap_of(zf, ZF_M2, [[4, 2], [1, 4]]),
                        ap_of(banks, S_KL * D + w * N, [[4 * D, 2], [D, 4], [1, s0]]),
                        axis=AX.X, op=OP.max,
                    )
                    v.tensor_tensor(
                        ap_of(zf, ZF_MU, [[1, 4]]),
                        ap_of(zf, ZF_M2, [[1, 4]]),
                        ap_of(zf, ZF_M2 + 4, [[1, 4]]),
                        OP.max,
                    )
                    # Ln range on ACT is +-2^64: compute via mu*2^-32
                    v.tensor_scalar_mul(
                        ap_of(zf, ZF_MU, [[1, 4]]), ap_of(zf, ZF_MU, [[1, 4]]), 2.0**-32
                    )
                    v.tensor_scalar_max(
                        ap_of(zf, ZF_MU, [[1, 4]]), ap_of(zf, ZF_MU, [[1, 4]]), 1e-36
                    )
                    sc.activation(
                        ap_of(zf, ZF_LM, [[1, 4]]), ap_of(zf, ZF_MU, [[1, 4]]), AF.Ln
                    )
                    # quantize the per-width shift to delta = -k*ln2 with k
                    # integer, so every rescale factor is an EXACT power of
                    # two (exact in bf16 as well).
                    # kf = round((log(mu*2^-32) + 32 ln2) / (w ln2))
                    v.tensor_scalar(
                        ap_of(zf, ZF_LM, [[1, 4]]), ap_of(zf, ZF_LM, [[1, 4]]),
                        LN2_32, 1.0 / (w * float(np.log(2.0))),
                        OP.add, OP.mult,
                    )
                    v.tensor_scalar(
                        ap_of(zf, ZF_LM, [[1, 4]]), ap_of(zf, ZF_LM, [[1, 4]]),
                        12582912.0, 12582912.0, OP.add, OP.subtract,
                    )
                    # dsum accumulates k (exact small integers)
                    v.tensor_tensor(
                        ap_of(zf, ZF_DSUM, [[1, 4]]),
                        ap_of(zf, ZF_DSUM, [[1, 4]]),
                        ap_of(zf, ZF_LM, [[1, 4]]),
                        OP.add,
                    )
                    # scale2 = 2^-k via exponent bits: (127 - k) << 23
                    v.tensor_scalar(
                        ap_of(zf, ZF_M2, [[1, 4]]), ap_of(zf, ZF_LM, [[1, 4]]),
                        -1.0, 127.0, OP.mult, OP.add,
                    )
                    zi = zf.bitcast(mybir.dt.int32)
                    v.tensor_copy(
                        ap_of(zi, ZF_M2 + 4, [[1, 4]]),
                        ap_of(zf, ZF_M2, [[1, 4]]),
                    )
                    v.tensor_scalar(
                        ap_of(zi, ZF_M2 + 4, [[1, 4]]),
                        ap_of(zi, ZF_M2 + 4, [[1, 4]]),
                        23, None, OP.arith_shift_left,
                    )
                    # M[g, d] = 2^(-k*d): d=0 -> 1, then multiplicative scan
                    v.memset(ap_of(zf, ZF_M, [[42, 4], [1, 1]]), 1.0)
                    for g in range(G):
                        sca = ap_of(zf, ZF_M2 + 4 + g, [[0, 41]])
                        v.tensor_tensor_scan(
                            ap_of(zf, ZF_M + g * 42 + 1, [[1, 41]]),
                            sca, sca, 1.0, OP.mult, OP.bypass,
                        )
                    # expand to Mx[g, d, i] = M[g, d] (packed bf16, exact
                    # powers of two) so the rescales hit the 2x DVE mode
                    v.tensor_copy(
                        ap_of(zb, ZB_MX, [[(w + 2) * N, 4], [N, w + 2], [1, N]]),
                        ap_of(zf, ZF_M, [[42, 4], [1, w + 2], [0, N]]),
                    )
                    # far const rows (needed from w=26 on) rescale on Pool,
                    # overlapping the DVE rescales and the next few widths
                    for g in range(G):
                        tCf = ap_of(consts, 4 * g * D + 26 * N,
                                    [[D, 4], [N, N - 26], [1, N]])
                        nc.gpsimd.tensor_tensor(
                            tCf, tCf,
                            ap_of(zb, ZB_MX + g * (w + 2) * N + N,
                                  [[0, 4], [0, N - 26], [1, N]]),
                            OP.mult,
                        )
                    for g in range(G):
                        mg = ZB_MX + g * (w + 2) * N
                        eng = nc.gpsimd if g == 3 else v
                        # natural tables, rows d<=w: scale by 2^(-k*d)
                        tA = ap_of(banks, g * D, [[4 * D, 4], [N, w + 1], [1, N]])
                        eng.tensor_tensor(
                            tA, tA,
                            ap_of(zb, mg, [[0, 4], [N, w + 1], [1, N]]),
                            OP.mult,
                        )
                        # IR/IL rows r<=w-1 hold width r+1: scale 2^(-k*(r+1))
                        tI = ap_of(banks, (16 + g) * D, [[4 * D, 2], [N, w], [1, N]])
                        eng.tensor_tensor(
                            tI, tI,
                            ap_of(zb, mg + N, [[0, 2], [N, w], [1, N]]),
                            OP.mult,
                        )
                        # near const rows w+1..25: one extra arc factor 2^-k
                        tC = ap_of(consts, 4 * g * D + (w + 1) * N,
                                   [[D, 4], [N, 25 - w], [1, N]])
                        v.tensor_tensor(
                            tC, tC,
                            ap_of(zb, mg + N, [[0, 4], [0, 25 - w], [1, N]]),
                            OP.mult,
                        )

            # ---- extract raw exp-domain CR[0, j] (log on host) ----
            v.tensor_copy(
                ap_of(zf, ZF_CROUT, [[N, 4], [1, N]]),
                ap_of(banks, S_CRA * D, [[D, 4], [N, N]]),
            )
            nc.sync.dma_start(
                ap_of(logs_d, 0, [[N, G], [1, N]], lead=[G * N, 128]),
                ap_of(zf, ZF_CROUT, [[N, G], [1, N]]),
            )
            nc.sync.dma_start(
                ap_of(dsum_d, 0, [[1, G]], lead=[G, 128]),
                ap_of(zf, ZF_DSUM, [[1, G]]),
            )

    nc.compile()
    return nc


_NC_CACHE = {}


def get_nc():
    if "nc" not in _NC_CACHE:
        _NC_CACHE["nc"] = build_nc()
    return _NC_CACHE["nc"]


def make_in_maps(trans_scores, dec_scores):
    t = np.asarray(trans_scores, dtype=np.float32)
    dec = np.asarray(dec_scores, dtype=np.float32)
    B = t.shape[0]
    go = dec[..., 0]                        # [B, n, dir, dv]
    # per-sentence linear pre-shift: each arc factor carries exp(-c0), so a
    # width-w entry is scaled exp(-c0*w); undone on the host at the end.
    tm = np.where(t < -1e8, -np.inf, t).max(axis=3)
    with np.errstate(invalid="ignore"):
        colmax = tm.max(axis=1)             # [B, n] best arc into each child
        proxy = np.nanmean(
            np.where(np.isfinite(colmax), colmax, np.nan)[:, 1:], axis=-1)
    c0 = (proxy + 0.5).astype(np.float32)
    c0 = np.clip(np.nan_to_num(c0), -20.0, 20.0)
    # one exp over trans (NEG -> 0 underflow is intended), then gather diags
    with np.errstate(under="ignore"):
        E = np.exp(t - c0[:, None, None, None])      # [B, n, n, 2]
        ego = np.exp(go)                             # [B, n, 2, 2]
    d_idx, i_idx = np.meshgrid(np.arange(N), np.arange(N), indexing="ij")
    j_idx = np.minimum(i_idx + d_idx, N - 1)
    valid = ((i_idx + d_idx) <= N - 1)[None].astype(np.float32)
    ea = E[:, i_idx, j_idx, :]              # [B, n, n, 2]  trans[i, i+d, v]
    eb = E[:, j_idx, i_idx, :]              # [B, n, n, 2]  trans[i+d, i, v]
    a1 = ea[..., 1] * ego[:, :, 1, 1][:, i_idx] * valid
    a0 = ea[..., 0] * ego[:, :, 1, 0][:, i_idx] * valid
    b1 = eb[..., 1] * ego[:, :, 0, 1][:, j_idx] * valid
    b0 = eb[..., 0] * ego[:, :, 0, 0][:, j_idx] * valid
    consts = np.empty((B, 4, N, N), dtype=np.float32)
    consts[:, 0] = a1
    consts[:, 1] = b1
    consts[:, 2] = a0
    consts[:, 3] = b0
    consts = consts.reshape(B, CONST_IN).astype(ml_dtypes.bfloat16)
    est = np.exp(dec[..., 1])               # [B, n, dir, dv]
    stops = np.empty((B, 4, N), dtype=np.float32)
    stops[:, 0] = est[:, :, 0, 0]; stops[:, 1] = est[:, :, 0, 1]
    stops[:, 2] = est[:, :, 1, 0]; stops[:, 3] = est[:, :, 1, 1]
    stops = stops.reshape(B, STOP_IN).astype(ml_dtypes.bfloat16)
    in_maps = []
    for c in range(NCORES):
        sl = slice(c * B_CORE, (c + 1) * B_CORE)
        in_maps.append({
            "consts": consts[sl],
            "stops": stops[sl],
        })
    return in_maps, c0


L0_HOST = 5  # sentences with len <= L0_HOST are computed exactly on the host


def _host_short_ll(trans, dec, lens):
    """Exact f64 LL for short sentences via the inside DP truncated to
    positions 0..L0_HOST (spans of a length-l sentence live within [0, l])."""
    n = L0_HOST + 1
    t = np.asarray(trans)[:, :n, :n, :].astype(np.float64)
    dc = np.asarray(dec)[:, :n].astype(np.float64)
    B = t.shape[0]
    go = dc[..., 0]
    stop = dc[..., 1]
    NEG = -1e9
    IR = np.full((B, n, n), NEG)
    IL = np.full((B, n, n), NEG)
    KR = np.full((B, n, n), NEG)
    KL = np.full((B, n, n), NEG)
    dg = np.arange(n)
    KR[:, dg, dg] = 0.0
    KL[:, dg, dg] = 0.0
    CR = np.full((B, n, n), NEG)
    CL = np.full((B, n, n), NEG)
    CR[:, dg, dg] = stop[:, :, 1, 0]
    CL[:, dg, dg] = stop[:, :, 0, 0]
    goR = go[:, :, 1, :]
    goL = go[:, :, 0, :]

    def lse(x):
        m = x.max(axis=-1, keepdims=True)
        return np.squeeze(m, -1) + np.log(np.exp(x - m).sum(axis=-1))

    for w in range(1, n):
        s = n - w
        i = np.arange(s)[:, None]
        tt = np.arange(w)[None, :]
        j = i + w
        k = i + tt
        vR = (tt > 0).astype(np.int64)
        ir = lse(KR[:, i, k] + goR[:, i, vR] + t[:, i, j, np.minimum(vR, 1)]
                 + CL[:, k + 1, j])
        vL = (tt < w - 1).astype(np.int64)
        il = lse(CR[:, i, k] + KL[:, k + 1, j] + goL[:, j, vL]
                 + t[:, j, i, np.minimum(vL, 1)])
        i1 = np.arange(s)
        IR[:, i1, i1 + w] = ir
        IL[:, i1, i1 + w] = il
        kr = lse(IR[:, i, i + 1 + tt] + CR[:, i + 1 + tt, j])
        kl = lse(CL[:, i, i + tt] + IL[:, i + tt, j])
        KR[:, i1, i1 + w] = kr
        KL[:, i1, i1 + w] = kl
        CR[:, i1, i1 + w] = kr + stop[:, i1, 1, 1]
        CL[:, i1, i1 + w] = kl + stop[:, i1 + w, 0, 1]

    return CR[np.arange(B), 0, lens].astype(np.float32)


def assemble(results, len_array, c0):
    ln = np.asarray(len_array).astype(np.int64)
    c0 = np.asarray(c0).astype(np.float64)
    out = np.empty(len(ln), dtype=np.float32)
    for c, res in enumerate(results):
        ecr = res["ecr"].reshape(B_CORE, N).astype(np.float64)
        dsum = res["dsum"].reshape(B_CORE).astype(np.float64)
        lc = ln[c * B_CORE:(c + 1) * B_CORE]
        idx = np.arange(B_CORE)
        with np.errstate(divide="ignore"):
            out[c * B_CORE:(c + 1) * B_CORE] = (
                np.log(ecr[idx, lc]) + dsum * np.log(2.0) * lc
                + c0[c * B_CORE:(c + 1) * B_CORE] * lc
            ).astype(np.float32)
    return out


def kernel(trans_scores, dec_scores, len_array):
    from concourse.bass_utils import run_bass_kernel_spmd

    nc = get_nc()
    in_maps, c0 = make_in_maps(trans_scores, dec_scores)
    res = run_bass_kernel_spmd(nc, in_maps, core_ids=list(range(NCORES)))
    out = assemble(res.results, len_array, c0)
    lens = np.asarray(len_array).astype(np.int64)
    short = lens <= L0_HOST
    if short.any():
        out[short] = _host_short_ll(
            np.asarray(trans_scores)[short], np.asarray(dec_scores)[short],
            lens[short])
    return out


# revision 24
# speedup vs baseline: 1.0369x; 1.0369x over previous
"""DMV inside algorithm (Eisner chart DP, logsumexp semiring) on Trainium2.

Strategy
--------
Pure data parallelism over the batch: 4096 sentences -> 8 cores x 512.
Per core: ONE run of 512 sentences laid out as [128 SBUF partitions] x
[G=4 sentence groups in the free dim], all chart tables in bf16.

The DP runs in the *exp domain* (no per-split transcendentals): tables hold
exp(score). Each width-w update is one fused strided multiply (products
P[qg,t,i]) followed by an in-place folding tree of adds that reduces over
the split dim t. Everything iterates [qg, t, i] with i innermost and
stride 1, which (with bf16) hits the DVE 2x packed-16-bit mode; the tree
of tensor_tensor adds also runs at 2x, unlike InstTensorReduce which gets
no fast mode.

Tables are stored *diagonal-packed*: Xd[d*41 + i] = X[i, i+d], making
every gather in the width-w recurrence a constant-stride access pattern.
IR/IL are stored with row r holding width r+1 (IL column-shifted by +1)
so all four quantities' gathers share one AP shape.

Numerics: scale composes linearly in span width (every width-w entry has
exactly w arcs), so one on-device renormalization at w=20 multiplies row
d by an exact power of two 2^(-k*d) (k integer per sentence), keeping
everything in range (bf16 range == f32 range). k is returned per sentence
and undone on the host: LL = log(CR[0,len]) + k*ln2*len + c0*len.
"""

import os

os.environ.setdefault("JAX_PLATFORMS", "cpu")

import numpy as np
import ml_dtypes

import concourse.bass as bass  # noqa: F401  (registers engine classes)
import concourse.tile as tile
import bass_rust
from concourse import bacc, mybir

F32 = mybir.dt.float32
BF16 = mybir.dt.bfloat16
AF = mybir.ActivationFunctionType
OP = mybir.AluOpType
AX = mybir.AxisListType

N = 41              # fake_len (ROOT at 0)
D = 1681            # table pitch: N*N elements
G = 4               # sentence groups per partition
NCORES = 8
B_CORE = 128 * G    # 512
CONST_IN = 4 * D    # host sends 4 pre-exponentiated tables/sentence (bf16)
STOP_IN = 4 * N     # host sends 4 exp'd stop vectors/sentence (bf16)
RENORM_W = 20

# banks tile (bf16): 24 diag-packed tables, slot k at offset k*D.
# Since every DP multiply is a per-direction 4-slot op, no table needs a
# second copy; all paired-access strides stay within 16*D (ISA AP steps
# are 16-bit).
#   KL g0..g3: 0..3    KR: 4..7    CRa: 8..11   CLa: 12..15
#   IR: 16..19         IL: 20..23  (IL stored col+1)
S_KL, S_KR, S_CRA, S_CLA, S_IR, S_IL = 0, 4, 8, 12, 16, 20
# consts tile (bf16): 4 per-arc tables, g-major: offset (4*g + C)*D
# (A0/B0 kept verbatim, not as differences: all-positive arithmetic so
# bf16 never hits catastrophic cancellation)
C_A1, C_B1, C_A0, C_B0 = range(4)
# stops tile (bf16): 16 vectors of 41, offset (g*4 + v)*41
V_SLNO, V_SLHAS, V_SRNO, V_SRHAS = range(4)

# bf16 scratch tile element offsets
ZB_P = 0          # 3360: products [qg, t, i] (qg-stride = s*w, varies by w)
ZB_T1 = 3360      # 328: tmp1 [q, g, i]
ZB_T2 = 3688      # 328
ZB_SS = 4016      # 328: reduce-path sums [q, g, i]
ZB_MX = 4344      # 4*22*41: expanded renorm multiplier Mx[g, d, i] = 2^(-k_g*d)
ZB_TOTAL = 7952

# f32 scratch tile element offsets
ZF_M2 = 0         # 8
ZF_MU = 8         # 4
ZF_LM = 12        # 4 (reused for k)
ZF_M = 16         # 4*42: renorm multiplier table [g, 42]
ZF_CROUT = 184    # 4*41
ZF_DSUM = 348     # 4
ZF_TOTAL = 352

LN2_32 = 32.0 * float(np.log(2.0))

# from this width on, the g3 lane of opA-L / opB-R (mult + folds) runs on
# the otherwise-idle Pool engine; below it Pool's per-op launch overhead
# would make the Pool chain longer than the whole DVE op
POOL_W0 = 4


def ap_of(t, offset, dims, lead=None):
    """Build a raw AP on tile/dram ap `t`: [lead or t.ap[0]] + dims."""
    ap = t.copy()
    first = list(t.ap[0]) if lead is None else list(lead)
    ap.ap = bass_rust.VecI64Pair([first] + [list(d) for d in dims])
    ap.offset = offset
    return ap


def build_nc():
    nc = bacc.Bacc("TRN2", target_bir_lowering=False, debug=False, num_devices=1)
    consts_in = nc.dram_tensor("consts", [B_CORE, CONST_IN], BF16, kind="ExternalInput").ap()
    stops_in = nc.dram_tensor("stops", [B_CORE, STOP_IN], BF16, kind="ExternalInput").ap()
    logs_d = nc.dram_tensor("ecr", [B_CORE, N], F32, kind="ExternalOutput").ap()
    dsum_d = nc.dram_tensor("dsum", [B_CORE], F32, kind="ExternalOutput").ap()

    with tile.TileContext(nc) as tc:
        with tc.tile_pool(name="p", bufs=1) as pool, \
                nc.allow_low_precision(reason="bf16 chart DP by design"):
            banks_t = pool.tile([128, 24 * D], BF16)
            consts_t = pool.tile([128, 16 * D], BF16)
            stops_t = pool.tile([128, 16 * N], BF16)
            zb_t = pool.tile([128, ZB_TOTAL], BF16)
            zf_t = pool.tile([128, ZF_TOTAL], F32)
            banks = banks_t[:]
            consts = consts_t[:]
            stops = stops_t[:]
            zb = zb_t[:]
            zf = zf_t[:]

            v = nc.vector
            sc = nc.scalar

            # ---- load host-precomputed exp-domain constants ----
            nc.sync.dma_start(
                ap_of(stops, 0, [[STOP_IN, G], [1, STOP_IN]]),
                ap_of(stops_in, 0,
                      [[STOP_IN, G], [1, STOP_IN]], lead=[G * STOP_IN, 128]),
            )
            # consts split by row range: step w reads row w only, so later
            # chunks' DMA hides under early DP steps
            # chunks of >=7 rows: a chunk of one table is (hi-lo)*82B
            # contiguous in DRAM, and >=512B descriptors avoid the small-
            # descriptor DMA penalty
            for lo, hi in ((0, 7 * N), (7 * N, 14 * N), (14 * N, 22 * N),
                           (22 * N, 31 * N), (31 * N, D)):
                nc.sync.dma_start(
                    ap_of(consts, lo, [[4 * D, G], [D, 4], [1, hi - lo]]),
                    ap_of(consts_in, lo, [[CONST_IN, G], [D, 4], [1, hi - lo]],
                          lead=[G * CONST_IN, 128]),
                )

            # ---- width-0 init ----
            # The renorm rescale reads full N-wide rows; the tail columns the
            # DP never writes hold whatever SBUF held before. That is safe:
            # those cells are only ever read by the renorm itself, which
            # multiplies them in place — garbage never reaches a DP-read cell,
            # so no zero-init is needed at all.
            v.memset(ap_of(zf, ZF_DSUM, [[1, G]]), 0.0)
            # KR[0,:] = KL[0,:] = 1
            v.memset(ap_of(banks, S_KL * D, [[4 * D, 2], [D, 4], [1, N]]), 1.0)
            # CRa[0,i] = exp(stop[i,R,NO]); CLa[0,i] = exp(stop[i,L,NO])
            v.tensor_copy(
                ap_of(banks, S_CRA * D, [[D, 4], [1, N]]),
                ap_of(stops, V_SRNO * N, [[4 * N, 4], [1, N]]),
            )
            v.tensor_copy(
                ap_of(banks, S_CLA * D, [[D, 4], [1, N]]),
                ap_of(stops, V_SLNO * N, [[4 * N, 4], [1, N]]),
            )

            def fold_chains(w, s, t0, count, chains):
                """In-place fold slot-chains of P[qg, t, i] over t in
                [t0, t0+count) down to one row at t0, interleaving the
                independent chains so they hide each other's semaphore
                latency. Each chain is (engine, first_slot, n_slots, final):
                `final` (if given) redirects that chain's last fold."""
                sw = s * w
                h = count
                while h > 1:
                    h2 = h // 2
                    hc = h - h2
                    for eng, slot0, nsl, fin in chains:
                        base = ZB_P + slot0 * sw + t0 * s
                        if hc == 1 and fin is not None:
                            out = fin
                        else:
                            out = ap_of(zb, base, [[sw, nsl], [s, h2], [1, s]])
                        eng.tensor_tensor(
                            out,
                            ap_of(zb, base, [[sw, nsl], [s, h2], [1, s]]),
                            ap_of(zb, base + hc * s, [[sw, nsl], [s, h2], [1, s]]),
                            OP.add,
                        )
                    h = hc

            # ---- chart DP ----
            for w in range(1, N):
                s = N - w
                sw = s * w
                row = (w - 1) * N + 1
                # opA: NOCHILD edges (t=0 for q=R, t=w-1 for q=L) have a
                # trivial K-factor of 1, so products cover only the w-1
                # HASCHILD splits; the edges are read straight from banks.
                # tmp2[q,g,i] = NOCHILD edge * {A0,B0}[w,:]
                v.tensor_tensor(
                    ap_of(zb, ZB_T2, [[4 * s, 2], [s, 4], [1, s]]),
                    ap_of(banks, S_CLA * D + (w - 1) * N + 1,
                          [[-4 * D - 1, 2], [D, 4], [1, s]]),
                    ap_of(consts, C_A0 * D + w * N, [[D, 2], [4 * D, 4], [1, s]]),
                    OP.mult,
                )
                if w > 1:
                    sw1 = s * (w - 1)
                    pool_w = POOL_W0 <= w
                    # q=R forward, t in [1,w): P[g,t,i] = KR[t,i]*CLa[w-1-t, i+t+1]
                    v.tensor_tensor(
                        ap_of(zb, ZB_P, [[sw1, 4], [s, w - 1], [1, s]]),
                        ap_of(banks, S_KR * D + N, [[D, 4], [N, w - 1], [1, s]]),
                        ap_of(banks, S_CLA * D + row - 40, [[D, 4], [-40, w - 1], [1, s]]),
                        OP.mult,
                    )
                    # q=L t-reversed, t' in [1,w): P[4+g,t',i] = CRa[w-1-t',i]*KL[t', i+w-t']
                    # (g3 of this half runs on the otherwise-idle Pool engine)
                    ng = 3 if pool_w else 4
                    v.tensor_tensor(
                        ap_of(zb, ZB_P + 4 * sw1, [[sw1, ng], [s, w - 1], [1, s]]),
                        ap_of(banks, S_CRA * D + (w - 2) * N, [[D, ng], [-N, w - 1], [1, s]]),
                        ap_of(banks, S_KL * D + N + w - 1, [[D, ng], [40, w - 1], [1, s]]),
                        OP.mult,
                    )
                    if pool_w:
                        nc.gpsimd.tensor_tensor(
                            ap_of(zb, ZB_P + 7 * sw1, [[sw1, 1], [s, w - 1], [1, s]]),
                            ap_of(banks, (S_CRA + 3) * D + (w - 2) * N, [[D, 1], [-N, w - 1], [1, s]]),
                            ap_of(banks, (S_KL + 3) * D + N + w - 1, [[D, 1], [40, w - 1], [1, s]]),
                            OP.mult,
                        )
                        fold_chains(w - 1, s, 0, w - 1, [
                            (v, 0, 4, None), (v, 4, 3, None), (nc.gpsimd, 7, 1, None)])
                    else:
                        fold_chains(w - 1, s, 0, w - 1, [(v, 0, 4, None), (v, 4, 4, None)])
                    # tmp1[q,g,i] = (sum over HASCHILD splits) * {A1,B1}[w,:]
                    v.tensor_tensor(
                        ap_of(zb, ZB_T1, [[4 * s, 2], [s, 4], [1, s]]),
                        ap_of(zb, ZB_P, [[4 * sw1, 2], [sw1, 4], [1, s]]),
                        ap_of(consts, C_A1 * D + w * N, [[D, 2], [4 * D, 4], [1, s]]),
                        OP.mult,
                    )
                    # IR[w-1, i] = tmp1R + tmp2R; IL[w-1, i+1] = tmp1L + tmp2L
                    # (two ops so each opB mult waits only on its own input)
                    v.tensor_tensor(
                        ap_of(banks, S_IR * D + (w - 1) * N, [[D, 4], [1, s]]),
                        ap_of(zb, ZB_T1, [[s, 4], [1, s]]),
                        ap_of(zb, ZB_T2, [[s, 4], [1, s]]),
                        OP.add,
                    )
                    v.tensor_tensor(
                        ap_of(banks, S_IL * D + (w - 1) * N + 1, [[D, 4], [1, s]]),
                        ap_of(zb, ZB_T1 + 4 * s, [[s, 4], [1, s]]),
                        ap_of(zb, ZB_T2 + 4 * s, [[s, 4], [1, s]]),
                        OP.add,
                    )
                else:
                    # w=1: only the NOCHILD edge exists: IR/IL row 0 = tmp2
                    v.tensor_copy(
                        ap_of(banks, S_IR * D, [[4 * D + 1, 2], [D, 4], [1, s]]),
                        ap_of(zb, ZB_T2, [[4 * s, 2], [s, 4], [1, s]]),
                    )
                # opB products, half 0 (q=L): P[g,t,i] = CLa[t,i]*IL[w-1-t, i+t+1]
                # half 1 (q=R): P[4+g,t,i] = IR[t,i]*CRa[w-1-t, i+t+1]
                klout = ap_of(banks, S_KL * D + w * N, [[D, 4], [1, s]])
                krout = ap_of(banks, S_KR * D + w * N, [[D, 4], [1, s]])
                if w == 1:
                    v.tensor_tensor(
                        klout,
                        ap_of(banks, S_CLA * D, [[D, 4], [N, 1], [1, s]]),
                        ap_of(banks, S_IL * D + row, [[D, 4], [-40, 1], [1, s]]),
                        OP.mult,
                    )
                    v.tensor_tensor(
                        krout,
                        ap_of(banks, S_IR * D, [[D, 4], [N, 1], [1, s]]),
                        ap_of(banks, S_CRA * D + row, [[D, 4], [-40, 1], [1, s]]),
                        OP.mult,
                    )
                else:
                    pool_w = POOL_W0 <= w
                    v.tensor_tensor(
                        ap_of(zb, ZB_P, [[sw, 4], [s, w], [1, s]]),
                        ap_of(banks, S_CLA * D, [[D, 4], [N, w], [1, s]]),
                        ap_of(banks, S_IL * D + row, [[D, 4], [-40, w], [1, s]]),
                        OP.mult,
                    )
                    ng = 3 if pool_w else 4
                    v.tensor_tensor(
                        ap_of(zb, ZB_P + 4 * sw, [[sw, ng], [s, w], [1, s]]),
                        ap_of(banks, S_IR * D, [[D, ng], [N, w], [1, s]]),
                        ap_of(banks, S_CRA * D + row, [[D, ng], [-40, w], [1, s]]),
                        OP.mult,
                    )
                    if pool_w:
                        nc.gpsimd.tensor_tensor(
                            ap_of(zb, ZB_P + 7 * sw, [[sw, 1], [s, w], [1, s]]),
                            ap_of(banks, (S_IR + 3) * D, [[D, 1], [N, w], [1, s]]),
                            ap_of(banks, (S_CRA + 3) * D + row, [[D, 1], [-40, w], [1, s]]),
                            OP.mult,
                        )
                        kr3 = ap_of(banks, (S_KR + 3) * D + w * N, [[D, 1], [1, s]])
                        krout3 = ap_of(banks, S_KR * D + w * N, [[D, 3], [1, s]])
                        fold_chains(w, s, 0, w, [
                            (v, 0, 4, klout), (v, 4, 3, krout3), (nc.gpsimd, 7, 1, kr3)])
                    else:
                        fold_chains(w, s, 0, w, [(v, 0, 4, klout), (v, 4, 4, krout)])
                # CRa[w,i] = KR[w,i]*sRhas[i]; CLa[w,i] = KL[w,i]*sLhas[i+w]
                v.tensor_tensor(
                    ap_of(banks, S_CRA * D + w * N, [[4 * D, 2], [D, 4], [1, s]]),
                    ap_of(banks, S_KR * D + w * N, [[-4 * D, 2], [D, 4], [1, s]]),
                    ap_of(stops, V_SRHAS * N, [[w - 2 * N, 2], [4 * N, 4], [1, s]]),
                    OP.mult,
                )

                if w == RENORM_W:
                    s0 = N - w
                    # mu[g] = max_i max(KR[w,i], KL[w,i])  (per partition)
                    v.tensor_reduce(
                        ap_of(zf, ZF_M2, [[4, 2], [1, 4]]),
                        ap_of(banks, S_KL * D + w * N, [[4 * D, 2], [D, 4], [1, s0]]),
                        axis=AX.X, op=OP.max,
                    )
                    v.tensor_tensor(
                        ap_of(zf, ZF_MU, [[1, 4]]),
                        ap_of(zf, ZF_M2, [[1, 4]]),
                        ap_of(zf, ZF_M2 + 4, [[1, 4]]),
                        OP.max,
                    )
                    # Ln range on ACT is +-2^64: compute via mu*2^-32
                    v.tensor_scalar_mul(
                        ap_of(zf, ZF_MU, [[1, 4]]), ap_of(zf, ZF_MU, [[1, 4]]), 2.0**-32
                    )
                    v.tensor_scalar_max(
                        ap_of(zf, ZF_MU, [[1, 4]]), ap_of(zf, ZF_MU, [[1, 4]]), 1e-36
                    )
                    sc.activation(
                        ap_of(zf, ZF_LM, [[1, 4]]), ap_of(zf, ZF_MU, [[1, 4]]), AF.Ln
                    )
                    # quantize the per-width shift to delta = -k*ln2 with k
                    # integer, so every rescale factor is an EXACT power of
                    # two (exact in bf16 as well).
                    # kf = round((log(mu*2^-32) + 32 ln2) / (w ln2))
                    v.tensor_scalar(
                        ap_of(zf, ZF_LM, [[1, 4]]), ap_of(zf, ZF_LM, [[1, 4]]),
                        LN2_32, 1.0 / (w * float(np.log(2.0))),
                        OP.add, OP.mult,
                    )
                    v.tensor_scalar(
                        ap_of(zf, ZF_LM, [[1, 4]]), ap_of(zf, ZF_LM, [[1, 4]]),
                        12582912.0, 12582912.0, OP.add, OP.subtract,
                    )
                    # dsum accumulates k (exact small integers)
                    v.tensor_tensor(
                        ap_of(zf, ZF_DSUM, [[1, 4]]),
                        ap_of(zf, ZF_DSUM, [[1, 4]]),
                        ap_of(zf, ZF_LM, [[1, 4]]),
                        OP.add,
                    )
                    # scale2 = 2^-k via exponent bits: (127 - k) << 23
                    v.tensor_scalar(
                        ap_of(zf, ZF_M2, [[1, 4]]), ap_of(zf, ZF_LM, [[1, 4]]),
                        -1.0, 127.0, OP.mult, OP.add,
                    )
                    zi = zf.bitcast(mybir.dt.int32)
                    v.tensor_copy(
                        ap_of(zi, ZF_M2 + 4, [[1, 4]]),
                        ap_of(zf, ZF_M2, [[1, 4]]),
                    )
                    v.tensor_scalar(
                        ap_of(zi, ZF_M2 + 4, [[1, 4]]),
                        ap_of(zi, ZF_M2 + 4, [[1, 4]]),
                        23, None, OP.arith_shift_left,
                    )
                    # M[g, d] = 2^(-k*d): d=0 -> 1, then multiplicative scan
                    v.memset(ap_of(zf, ZF_M, [[42, 4], [1, 1]]), 1.0)
                    for g in range(G):
                        sca = ap_of(zf, ZF_M2 + 4 + g, [[0, 41]])
                        v.tensor_tensor_scan(
                            ap_of(zf, ZF_M + g * 42 + 1, [[1, 41]]),
                            sca, sca, 1.0, OP.mult, OP.bypass,
                        )
                    # expand to Mx[g, d, i] = M[g, d] (packed bf16, exact
                    # powers of two) so the rescales hit the 2x DVE mode
                    v.tensor_copy(
                        ap_of(zb, ZB_MX, [[(w + 2) * N, 4], [N, w + 2], [1, N]]),
                        ap_of(zf, ZF_M, [[42, 4], [1, w + 2], [0, N]]),
                    )
                    # far const rows (needed from w=26 on) rescale on Pool,
                    # overlapping the DVE rescales and the next few widths
                    for g in range(G):
                        tCf = ap_of(consts, 4 * g * D + 26 * N,
                                    [[D, 4], [N, N - 26], [1, N]])
                        nc.gpsimd.tensor_tensor(
                            tCf, tCf,
                            ap_of(zb, ZB_MX + g * (w + 2) * N + N,
                                  [[0, 4], [0, N - 26], [1, N]]),
                            OP.mult,
                        )
                    for g in range(G):
                        mg = ZB_MX + g * (w + 2) * N
                        eng = v
                        # natural tables, rows d<=w: scale by 2^(-k*d)
                        tA = ap_of(banks, g * D, [[4 * D, 4], [N, w + 1], [1, N]])
                        eng.tensor_tensor(
                            tA, tA,
                            ap_of(zb, mg, [[0, 4], [N, w + 1], [1, N]]),
                            OP.mult,
                        )
                        # IR/IL rows r<=w-1 hold width r+1: scale 2^(-k*(r+1))
                        tI = ap_of(banks, (16 + g) * D, [[4 * D, 2], [N, w], [1, N]])
                        eng.tensor_tensor(
                            tI, tI,
                            ap_of(zb, mg + N, [[0, 2], [N, w], [1, N]]),
                            OP.mult,
                        )
                        # near const rows w+1..25: one extra arc factor 2^-k
                        tC = ap_of(consts, 4 * g * D + (w + 1) * N,
                                   [[D, 4], [N, 25 - w], [1, N]])
                        v.tensor_tensor(
                            tC, tC,
                            ap_of(zb, mg + N, [[0, 4], [0, 25 - w], [1, N]]),
                            OP.mult,
                        )

            # ---- extract raw exp-domain CR[0, j] (log on host) ----
            v.tensor_copy(
                ap_of(zf, ZF_CROUT, [[N, 4], [1, N]]),
                ap_of(banks, S_CRA * D, [[D, 4], [N, N]]),
            )
            nc.sync.dma_start(
                ap_of(logs_d, 0, [[N, G], [1, N]], lead=[G * N, 128]),
                ap_of(zf, ZF_CROUT, [[N, G], [1, N]]),
            )
            nc.sync.dma_start(
                ap_of(dsum_d, 0, [[1, G]], lead=[G, 128]),
                ap_of(zf, ZF_DSUM, [[1, G]]),
            )

    nc.compile()
    return nc


_NC_CACHE = {}


def get_nc():
    if "nc" not in _NC_CACHE:
        _NC_CACHE["nc"] = build_nc()
    return _NC_CACHE["nc"]


def make_in_maps(trans_scores, dec_scores):
    t = np.asarray(trans_scores, dtype=np.float32)
    dec = np.asarray(dec_scores, dtype=np.float32)
    B = t.shape[0]
    go = dec[..., 0]                        # [B, n, dir, dv]
    # per-sentence linear pre-shift: each arc factor carries exp(-c0), so a
    # width-w entry is scaled exp(-c0*w); undone on the host at the end.
    tm = np.where(t < -1e8, -np.inf, t).max(axis=3)
    with np.errstate(invalid="ignore"):
        colmax = tm.max(axis=1)             # [B, n] best arc into each child
        proxy = np.nanmean(
            np.where(np.isfinite(colmax), colmax, np.nan)[:, 1:], axis=-1)
    c0 = (proxy + 0.5).astype(np.float32)
    c0 = np.clip(np.nan_to_num(c0), -20.0, 20.0)
    # one exp over trans (NEG -> 0 underflow is intended), then gather diags
    with np.errstate(under="ignore"):
        E = np.exp(t - c0[:, None, None, None])      # [B, n, n, 2]
        ego = np.exp(go)                             # [B, n, 2, 2]
    d_idx, i_idx = np.meshgrid(np.arange(N), np.arange(N), indexing="ij")
    j_idx = np.minimum(i_idx + d_idx, N - 1)
    valid = ((i_idx + d_idx) <= N - 1)[None].astype(np.float32)
    ea = E[:, i_idx, j_idx, :]              # [B, n, n, 2]  trans[i, i+d, v]
    eb = E[:, j_idx, i_idx, :]              # [B, n, n, 2]  trans[i+d, i, v]
    a1 = ea[..., 1] * ego[:, :, 1, 1][:, i_idx] * valid
    a0 = ea[..., 0] * ego[:, :, 1, 0][:, i_idx] * valid
    b1 = eb[..., 1] * ego[:, :, 0, 1][:, j_idx] * valid
    b0 = eb[..., 0] * ego[:, :, 0, 0][:, j_idx] * valid
    consts = np.empty((B, 4, N, N), dtype=np.float32)
    consts[:, 0] = a1
    consts[:, 1] = b1
    consts[:, 2] = a0
    consts[:, 3] = b0
    consts = consts.reshape(B, CONST_IN).astype(ml_dtypes.bfloat16)
    est = np.exp(dec[..., 1])               # [B, n, dir, dv]
    stops = np.empty((B, 4, N), dtype=np.float32)
    stops[:, 0] = est[:, :, 0, 0]; stops[:, 1] = est[:, :, 0, 1]
    stops[:, 2] = est[:, :, 1, 0]; stops[:, 3] = est[:, :, 1, 1]
    stops = stops.reshape(B, STOP_IN).astype(ml_dtypes.bfloat16)
    in_maps = []
    for c in range(NCORES):
        sl = slice(c * B_CORE, (c + 1) * B_CORE)
        in_maps.append({
            "consts": consts[sl],
            "stops": stops[sl],
        })
    return in_maps, c0


L0_HOST = 5  # sentences with len <= L0_HOST are computed exactly on the host


def _host_short_ll(trans, dec, lens):
    """Exact f64 LL for short sentences via the inside DP truncated to
    positions 0..L0_HOST (spans of a length-l sentence live within [0, l])."""
    n = L0_HOST + 1
    t = np.asarray(trans)[:, :n, :n, :].astype(np.float64)
    dc = np.asarray(dec)[:, :n].astype(np.float64)
    B = t.shape[0]
    go = dc[..., 0]
    stop = dc[..., 1]
    NEG = -1e9
    IR = np.full((B, n, n), NEG)
    IL = np.full((B, n, n), NEG)
    KR = np.full((B, n, n), NEG)
    KL = np.full((B, n, n), NEG)
    dg = np.arange(n)
    KR[:, dg, dg] = 0.0
    KL[:, dg, dg] = 0.0
    CR = np.full((B, n, n), NEG)
    CL = np.full((B, n, n), NEG)
    CR[:, dg, dg] = stop[:, :, 1, 0]
    CL[:, dg, dg] = stop[:, :, 0, 0]
    goR = go[:, :, 1, :]
    goL = go[:, :, 0, :]

    def lse(x):
        m = x.max(axis=-1, keepdims=True)
        return np.squeeze(m, -1) + np.log(np.exp(x - m).sum(axis=-1))

    for w in range(1, n):
        s = n - w
        i = np.arange(s)[:, None]
        tt = np.arange(w)[None, :]
        j = i + w
        k = i + tt
        vR = (tt > 0).astype(np.int64)
        ir = lse(KR[:, i, k] + goR[:, i, vR] + t[:, i, j, np.minimum(vR, 1)]
                 + CL[:, k + 1, j])
        vL = (tt < w - 1).astype(np.int64)
        il = lse(CR[:, i, k] + KL[:, k + 1, j] + goL[:, j, vL]
                 + t[:, j, i, np.minimum(vL, 1)])
        i1 = np.arange(s)
        IR[:, i1, i1 + w] = ir
        IL[:, i1, i1 + w] = il
        kr = lse(IR[:, i, i + 1 + tt] + CR[:, i + 1 + tt, j])
        kl = lse(CL[:, i, i + tt] + IL[:, i + tt, j])
        KR[:, i1, i1 + w] = kr
        KL[:, i1, i1 + w] = kl
        CR[:, i1, i1 + w] = kr + stop[:, i1, 1, 1]
        CL[:, i1, i1 + w] = kl + stop[:, i1 + w, 0, 1]

    return CR[np.arange(B), 0, lens].astype(np.float32)


def assemble(results, len_array, c0):
    ln = np.asarray(len_array).astype(np.int64)
    c0 = np.asarray(c0).astype(np.float64)
    out = np.empty(len(ln), dtype=np.float32)
    for c, res in enumerate(results):
        ecr = res["ecr"].reshape(B_CORE, N).astype(np.float64)
        dsum = res["dsum"].reshape(B_CORE).astype(np.float64)
        lc = ln[c * B_CORE:(c + 1) * B_CORE]
        idx = np.arange(B_CORE)
        with np.errstate(divide="ignore"):
            out[c * B_CORE:(c + 1) * B_CORE] = (
                np.log(ecr[idx, lc]) + dsum * np.log(2.0) * lc
                + c0[c * B_CORE:(c + 1) * B_CORE] * lc
            ).astype(np.float32)
    return out


def kernel(trans_scores, dec_scores, len_array):
    from concourse.bass_utils import run_bass_kernel_spmd

    nc = get_nc()
    in_maps, c0 = make_in_maps(trans_scores, dec_scores)
    res = run_bass_kernel_spmd(nc, in_maps, core_ids=list(range(NCORES)))
    out = assemble(res.results, len_array, c0)
    lens = np.asarray(len_array).astype(np.int64)
    short = lens <= L0_HOST
    if short.any():
        out[short] = _host_short_ll(
            np.asarray(trans_scores)[short], np.asarray(dec_scores)[short],
            lens[short])
    return out


# revision 26
# speedup vs baseline: 1.0382x; 1.0013x over previous
"""DMV inside algorithm (Eisner chart DP, logsumexp semiring) on Trainium2.

Strategy
--------
Pure data parallelism over the batch: 4096 sentences -> 8 cores x 512.
Per core: ONE run of 512 sentences laid out as [128 SBUF partitions] x
[G=4 sentence groups in the free dim], all chart tables in bf16.

The DP runs in the *exp domain* (no per-split transcendentals): tables hold
exp(score). Each width-w update is one fused strided multiply (products
P[qg,t,i]) followed by an in-place folding tree of adds that reduces over
the split dim t. Everything iterates [qg, t, i] with i innermost and
stride 1, which (with bf16) hits the DVE 2x packed-16-bit mode; the tree
of tensor_tensor adds also runs at 2x, unlike InstTensorReduce which gets
no fast mode.

Tables are stored *diagonal-packed*: Xd[d*41 + i] = X[i, i+d], making
every gather in the width-w recurrence a constant-stride access pattern.
IR/IL are stored with row r holding width r+1 (IL column-shifted by +1)
so all four quantities' gathers share one AP shape.

Numerics: scale composes linearly in span width (every width-w entry has
exactly w arcs), so one on-device renormalization at w=20 multiplies row
d by an exact power of two 2^(-k*d) (k integer per sentence), keeping
everything in range (bf16 range == f32 range). k is returned per sentence
and undone on the host: LL = log(CR[0,len]) + k*ln2*len + c0*len.
"""

import os

os.environ.setdefault("JAX_PLATFORMS", "cpu")

import numpy as np
import ml_dtypes

import concourse.bass as bass  # noqa: F401  (registers engine classes)
import concourse.tile as tile
import bass_rust
from concourse import bacc, mybir

F32 = mybir.dt.float32
BF16 = mybir.dt.bfloat16
AF = mybir.ActivationFunctionType
OP = mybir.AluOpType
AX = mybir.AxisListType

N = 41              # fake_len (ROOT at 0)
D = 1681            # table pitch: N*N elements
G = 4               # sentence groups per partition
NCORES = 8
B_CORE = 128 * G    # 512
CONST_IN = 4 * D    # host sends 4 pre-exponentiated tables/sentence (bf16)
STOP_IN = 4 * N     # host sends 4 exp'd stop vectors/sentence (bf16)
RENORM_W = 20

# banks tile (bf16): 24 diag-packed tables, slot k at offset k*D.
# Since every DP multiply is a per-direction 4-slot op, no table needs a
# second copy; all paired-access strides stay within 16*D (ISA AP steps
# are 16-bit).
#   KL g0..g3: 0..3    KR: 4..7    CRa: 8..11   CLa: 12..15
#   IR: 16..19         IL: 20..23  (IL stored col+1)
S_KL, S_KR, S_CRA, S_CLA, S_IR, S_IL = 0, 4, 8, 12, 16, 20
# consts tile (bf16): 4 per-arc tables, g-major: offset (4*g + C)*D
# (A0/B0 kept verbatim, not as differences: all-positive arithmetic so
# bf16 never hits catastrophic cancellation)
C_A1, C_B1, C_A0, C_B0 = range(4)
# stops tile (bf16): 16 vectors of 41, offset (g*4 + v)*41
V_SLNO, V_SLHAS, V_SRNO, V_SRHAS = range(4)

# bf16 scratch tile element offsets
ZB_P = 0          # 3360: products [qg, t, i] (qg-stride = s*w, varies by w)
ZB_T1 = 3360      # 328: tmp1 [q, g, i]
ZB_T2 = 3688      # 328
ZB_SS = 4016      # 328: reduce-path sums [q, g, i]
ZB_MX = 4344      # 4*22*41: expanded renorm multiplier Mx[g, d, i] = 2^(-k_g*d)
ZB_TOTAL = 7952

# f32 scratch tile element offsets
ZF_M2 = 0         # 8
ZF_MU = 8         # 4
ZF_LM = 12        # 4 (reused for k)
ZF_M = 16         # 4*42: renorm multiplier table [g, 42]
ZF_CROUT = 184    # 4*41
ZF_DSUM = 348     # 4
ZF_TOTAL = 352

LN2_32 = 32.0 * float(np.log(2.0))

# from this width on, the g3 lane of opA-L / opB-R (mult + folds) runs on
# the otherwise-idle Pool engine; below it Pool's per-op launch overhead
# would make the Pool chain longer than the whole DVE op
POOL_W0 = 4
POOL2_LO, POOL2_HI = 99, 0  # widths with two Pool lanes (off by default)


def ap_of(t, offset, dims, lead=None):
    """Build a raw AP on tile/dram ap `t`: [lead or t.ap[0]] + dims."""
    ap = t.copy()
    first = list(t.ap[0]) if lead is None else list(lead)
    ap.ap = bass_rust.VecI64Pair([first] + [list(d) for d in dims])
    ap.offset = offset
    return ap


def build_nc():
    nc = bacc.Bacc("TRN2", target_bir_lowering=False, debug=False, num_devices=1)
    consts_in = nc.dram_tensor("consts", [B_CORE, CONST_IN], BF16, kind="ExternalInput").ap()
    stops_in = nc.dram_tensor("stops", [B_CORE, STOP_IN], BF16, kind="ExternalInput").ap()
    logs_d = nc.dram_tensor("ecr", [B_CORE, N], F32, kind="ExternalOutput").ap()
    dsum_d = nc.dram_tensor("dsum", [B_CORE], F32, kind="ExternalOutput").ap()

    with tile.TileContext(nc) as tc:
        with tc.tile_pool(name="p", bufs=1) as pool, \
                nc.allow_low_precision(reason="bf16 chart DP by design"):
            banks_t = pool.tile([128, 24 * D], BF16)
            consts_t = pool.tile([128, 16 * D], BF16)
            stops_t = pool.tile([128, 16 * N], BF16)
            zb_t = pool.tile([128, ZB_TOTAL], BF16)
            zf_t = pool.tile([128, ZF_TOTAL], F32)
            banks = banks_t[:]
            consts = consts_t[:]
            stops = stops_t[:]
            zb = zb_t[:]
            zf = zf_t[:]

            v = nc.vector
            sc = nc.scalar

            # ---- load host-precomputed exp-domain constants ----
            nc.sync.dma_start(
                ap_of(stops, 0, [[STOP_IN, G], [1, STOP_IN]]),
                ap_of(stops_in, 0,
                      [[STOP_IN, G], [1, STOP_IN]], lead=[G * STOP_IN, 128]),
            )
            # consts split by row range: step w reads row w only, so later
            # chunks' DMA hides under early DP steps
            # chunks of >=7 rows: a chunk of one table is (hi-lo)*82B
            # contiguous in DRAM, and >=512B descriptors avoid the small-
            # descriptor DMA penalty
            for lo, hi in ((0, 3 * N), (3 * N, 10 * N), (10 * N, 18 * N),
                           (18 * N, 26 * N), (26 * N, 34 * N), (34 * N, D)):
                nc.sync.dma_start(
                    ap_of(consts, lo, [[4 * D, G], [D, 4], [1, hi - lo]]),
                    ap_of(consts_in, lo, [[CONST_IN, G], [D, 4], [1, hi - lo]],
                          lead=[G * CONST_IN, 128]),
                )

            # ---- width-0 init ----
            # The renorm rescale reads full N-wide rows; the tail columns the
            # DP never writes hold whatever SBUF held before. That is safe:
            # those cells are only ever read by the renorm itself, which
            # multiplies them in place — garbage never reaches a DP-read cell,
            # so no zero-init is needed at all.
            v.memset(ap_of(zf, ZF_DSUM, [[1, G]]), 0.0)
            # KR[0,:] = KL[0,:] = 1
            v.memset(ap_of(banks, S_KL * D, [[4 * D, 2], [D, 4], [1, N]]), 1.0)
            # CRa[0,i] = exp(stop[i,R,NO]); CLa[0,i] = exp(stop[i,L,NO])
            v.tensor_copy(
                ap_of(banks, S_CRA * D, [[D, 4], [1, N]]),
                ap_of(stops, V_SRNO * N, [[4 * N, 4], [1, N]]),
            )
            v.tensor_copy(
                ap_of(banks, S_CLA * D, [[D, 4], [1, N]]),
                ap_of(stops, V_SLNO * N, [[4 * N, 4], [1, N]]),
            )

            def fold_chains(w, s, t0, count, chains):
                """In-place fold slot-chains of P[qg, t, i] over t in
                [t0, t0+count) down to one row at t0, interleaving the
                independent chains so they hide each other's semaphore
                latency. Each chain is (engine, first_slot, n_slots, final):
                `final` (if given) redirects that chain's last fold."""
                sw = s * w
                h = count
                while h > 1:
                    h2 = h // 2
                    hc = h - h2
                    for eng, slot0, nsl, fin in chains:
                        base = ZB_P + slot0 * sw + t0 * s
                        if hc == 1 and fin is not None:
                            out = fin
                        else:
                            out = ap_of(zb, base, [[sw, nsl], [s, h2], [1, s]])
                        eng.tensor_tensor(
                            out,
                            ap_of(zb, base, [[sw, nsl], [s, h2], [1, s]]),
                            ap_of(zb, base + hc * s, [[sw, nsl], [s, h2], [1, s]]),
                            OP.add,
                        )
                    h = hc

            # ---- chart DP ----
            for w in range(1, N):
                s = N - w
                sw = s * w
                row = (w - 1) * N + 1
                # opA: NOCHILD edges (t=0 for q=R, t=w-1 for q=L) have a
                # trivial K-factor of 1, so products cover only the w-1
                # HASCHILD splits; the edges are read straight from banks.
                # tmp2[q,g,i] = NOCHILD edge * {A0,B0}[w,:]
                v.tensor_tensor(
                    ap_of(zb, ZB_T2, [[4 * s, 2], [s, 4], [1, s]]),
                    ap_of(banks, S_CLA * D + (w - 1) * N + 1,
                          [[-4 * D - 1, 2], [D, 4], [1, s]]),
                    ap_of(consts, C_A0 * D + w * N, [[D, 2], [4 * D, 4], [1, s]]),
                    OP.mult,
                )
                if w > 1:
                    sw1 = s * (w - 1)
                    npl = 0 if w < POOL_W0 else (2 if POOL2_LO <= w <= POOL2_HI else 1)
                    # q=R forward, t in [1,w): P[g,t,i] = KR[t,i]*CLa[w-1-t, i+t+1]
                    v.tensor_tensor(
                        ap_of(zb, ZB_P, [[sw1, 4], [s, w - 1], [1, s]]),
                        ap_of(banks, S_KR * D + N, [[D, 4], [N, w - 1], [1, s]]),
                        ap_of(banks, S_CLA * D + row - 40, [[D, 4], [-40, w - 1], [1, s]]),
                        OP.mult,
                    )
                    # q=L t-reversed, t' in [1,w): P[4+g,t',i] = CRa[w-1-t',i]*KL[t', i+w-t']
                    # (trailing g-lanes of this half run on the idle Pool engine)
                    ng = 4 - npl
                    v.tensor_tensor(
                        ap_of(zb, ZB_P + 4 * sw1, [[sw1, ng], [s, w - 1], [1, s]]),
                        ap_of(banks, S_CRA * D + (w - 2) * N, [[D, ng], [-N, w - 1], [1, s]]),
                        ap_of(banks, S_KL * D + N + w - 1, [[D, ng], [40, w - 1], [1, s]]),
                        OP.mult,
                    )
                    if npl:
                        nc.gpsimd.tensor_tensor(
                            ap_of(zb, ZB_P + (4 + ng) * sw1, [[sw1, npl], [s, w - 1], [1, s]]),
                            ap_of(banks, (S_CRA + ng) * D + (w - 2) * N, [[D, npl], [-N, w - 1], [1, s]]),
                            ap_of(banks, (S_KL + ng) * D + N + w - 1, [[D, npl], [40, w - 1], [1, s]]),
                            OP.mult,
                        )
                        fold_chains(w - 1, s, 0, w - 1, [
                            (v, 0, 4, None), (v, 4, ng, None), (nc.gpsimd, 4 + ng, npl, None)])
                    else:
                        fold_chains(w - 1, s, 0, w - 1, [(v, 0, 4, None), (v, 4, 4, None)])
                    # tmp1[q,g,i] = (sum over HASCHILD splits) * {A1,B1}[w,:]
                    v.tensor_tensor(
                        ap_of(zb, ZB_T1, [[4 * s, 2], [s, 4], [1, s]]),
                        ap_of(zb, ZB_P, [[4 * sw1, 2], [sw1, 4], [1, s]]),
                        ap_of(consts, C_A1 * D + w * N, [[D, 2], [4 * D, 4], [1, s]]),
                        OP.mult,
                    )
                    # IR[w-1, i] = tmp1R + tmp2R; IL[w-1, i+1] = tmp1L + tmp2L
                    # (two ops so each opB mult waits only on its own input)
                    v.tensor_tensor(
                        ap_of(banks, S_IR * D + (w - 1) * N, [[D, 4], [1, s]]),
                        ap_of(zb, ZB_T1, [[s, 4], [1, s]]),
                        ap_of(zb, ZB_T2, [[s, 4], [1, s]]),
                        OP.add,
                    )
                    v.tensor_tensor(
                        ap_of(banks, S_IL * D + (w - 1) * N + 1, [[D, 4], [1, s]]),
                        ap_of(zb, ZB_T1 + 4 * s, [[s, 4], [1, s]]),
                        ap_of(zb, ZB_T2 + 4 * s, [[s, 4], [1, s]]),
                        OP.add,
                    )
                else:
                    # w=1: only the NOCHILD edge exists: IR/IL row 0 = tmp2
                    v.tensor_copy(
                        ap_of(banks, S_IR * D, [[4 * D + 1, 2], [D, 4], [1, s]]),
                        ap_of(zb, ZB_T2, [[4 * s, 2], [s, 4], [1, s]]),
                    )
                # opB products, half 0 (q=L): P[g,t,i] = CLa[t,i]*IL[w-1-t, i+t+1]
                # half 1 (q=R): P[4+g,t,i] = IR[t,i]*CRa[w-1-t, i+t+1]
                klout = ap_of(banks, S_KL * D + w * N, [[D, 4], [1, s]])
                krout = ap_of(banks, S_KR * D + w * N, [[D, 4], [1, s]])
                if w == 1:
                    v.tensor_tensor(
                        klout,
                        ap_of(banks, S_CLA * D, [[D, 4], [N, 1], [1, s]]),
                        ap_of(banks, S_IL * D + row, [[D, 4], [-40, 1], [1, s]]),
                        OP.mult,
                    )
                    v.tensor_tensor(
                        krout,
                        ap_of(banks, S_IR * D, [[D, 4], [N, 1], [1, s]]),
                        ap_of(banks, S_CRA * D + row, [[D, 4], [-40, 1], [1, s]]),
                        OP.mult,
                    )
                else:
                    npl = 0 if w < POOL_W0 else (2 if POOL2_LO <= w <= POOL2_HI else 1)
                    v.tensor_tensor(
                        ap_of(zb, ZB_P, [[sw, 4], [s, w], [1, s]]),
                        ap_of(banks, S_CLA * D, [[D, 4], [N, w], [1, s]]),
                        ap_of(banks, S_IL * D + row, [[D, 4], [-40, w], [1, s]]),
                        OP.mult,
                    )
                    ng = 4 - npl
                    v.tensor_tensor(
                        ap_of(zb, ZB_P + 4 * sw, [[sw, ng], [s, w], [1, s]]),
                        ap_of(banks, S_IR * D, [[D, ng], [N, w], [1, s]]),
                        ap_of(banks, S_CRA * D + row, [[D, ng], [-40, w], [1, s]]),
                        OP.mult,
                    )
                    if npl:
                        nc.gpsimd.tensor_tensor(
                            ap_of(zb, ZB_P + (4 + ng) * sw, [[sw, npl], [s, w], [1, s]]),
                            ap_of(banks, (S_IR + ng) * D, [[D, npl], [N, w], [1, s]]),
                            ap_of(banks, (S_CRA + ng) * D + row, [[D, npl], [-40, w], [1, s]]),
                            OP.mult,
                        )
                        krp = ap_of(banks, (S_KR + ng) * D + w * N, [[D, npl], [1, s]])
                        krd = ap_of(banks, S_KR * D + w * N, [[D, ng], [1, s]])
                        fold_chains(w, s, 0, w, [
                            (v, 0, 4, klout), (v, 4, ng, krd), (nc.gpsimd, 4 + ng, npl, krp)])
                    else:
                        fold_chains(w, s, 0, w, [(v, 0, 4, klout), (v, 4, 4, krout)])
                # CRa[w,i] = KR[w,i]*sRhas[i]; CLa[w,i] = KL[w,i]*sLhas[i+w]
                v.tensor_tensor(
                    ap_of(banks, S_CRA * D + w * N, [[4 * D, 2], [D, 4], [1, s]]),
                    ap_of(banks, S_KR * D + w * N, [[-4 * D, 2], [D, 4], [1, s]]),
                    ap_of(stops, V_SRHAS * N, [[w - 2 * N, 2], [4 * N, 4], [1, s]]),
                    OP.mult,
                )

                if w == RENORM_W:
                    s0 = N - w
                    # mu[g] = max_i max(KR[w,i], KL[w,i])  (per partition)
                    v.tensor_reduce(
                        ap_of(zf, ZF_M2, [[4, 2], [1, 4]]),
                        ap_of(banks, S_KL * D + w * N, [[4 * D, 2], [D, 4], [1, s0]]),
                        axis=AX.X, op=OP.max,
                    )
                    v.tensor_tensor(
                        ap_of(zf, ZF_MU, [[1, 4]]),
                        ap_of(zf, ZF_M2, [[1, 4]]),
                        ap_of(zf, ZF_M2 + 4, [[1, 4]]),
                        OP.max,
                    )
                    # Ln range on ACT is +-2^64: compute via mu*2^-32
                    v.tensor_scalar_mul(
                        ap_of(zf, ZF_MU, [[1, 4]]), ap_of(zf, ZF_MU, [[1, 4]]), 2.0**-32
                    )
                    v.tensor_scalar_max(
                        ap_of(zf, ZF_MU, [[1, 4]]), ap_of(zf, ZF_MU, [[1, 4]]), 1e-36
                    )
                    sc.activation(
                        ap_of(zf, ZF_LM, [[1, 4]]), ap_of(zf, ZF_MU, [[1, 4]]), AF.Ln
                    )
                    # quantize the per-width shift to delta = -k*ln2 with k
                    # integer, so every rescale factor is an EXACT power of
                    # two (exact in bf16 as well).
                    # kf = round((log(mu*2^-32) + 32 ln2) / (w ln2))
                    v.tensor_scalar(
                        ap_of(zf, ZF_LM, [[1, 4]]), ap_of(zf, ZF_LM, [[1, 4]]),
                        LN2_32, 1.0 / (w * float(np.log(2.0))),
                        OP.add, OP.mult,
                    )
                    v.tensor_scalar(
                        ap_of(zf, ZF_LM, [[1, 4]]), ap_of(zf, ZF_LM, [[1, 4]]),
                        12582912.0, 12582912.0, OP.add, OP.subtract,
                    )
                    # dsum accumulates k (exact small integers)
                    v.tensor_tensor(
                        ap_of(zf, ZF_DSUM, [[1, 4]]),
                        ap_of(zf, ZF_DSUM, [[1, 4]]),
                        ap_of(zf, ZF_LM, [[1, 4]]),
                        OP.add,
                    )
                    # scale2 = 2^-k via exponent bits: (127 - k) << 23
                    v.tensor_scalar(
                        ap_of(zf, ZF_M2, [[1, 4]]), ap_of(zf, ZF_LM, [[1, 4]]),
                        -1.0, 127.0, OP.mult, OP.add,
                    )
                    zi = zf.bitcast(mybir.dt.int32)
                    v.tensor_copy(
                        ap_of(zi, ZF_M2 + 4, [[1, 4]]),
                        ap_of(zf, ZF_M2, [[1, 4]]),
                    )
                    v.tensor_scalar(
                        ap_of(zi, ZF_M2 + 4, [[1, 4]]),
                        ap_of(zi, ZF_M2 + 4, [[1, 4]]),
                        23, None, OP.arith_shift_left,
                    )
                    # M[g, d] = 2^(-k*d): d=0 -> 1, then multiplicative scan
                    v.memset(ap_of(zf, ZF_M, [[42, 4], [1, 1]]), 1.0)
                    for g in range(G):
                        sca = ap_of(zf, ZF_M2 + 4 + g, [[0, 41]])
                        v.tensor_tensor_scan(
                            ap_of(zf, ZF_M + g * 42 + 1, [[1, 41]]),
                            sca, sca, 1.0, OP.mult, OP.bypass,
                        )
                    # expand to Mx[g, d, i] = M[g, d] (packed bf16, exact
                    # powers of two) so the rescales hit the 2x DVE mode
                    v.tensor_copy(
                        ap_of(zb, ZB_MX, [[(w + 2) * N, 4], [N, w + 2], [1, N]]),
                        ap_of(zf, ZF_M, [[42, 4], [1, w + 2], [0, N]]),
                    )
                    # far const rows (needed from w=26 on) rescale on Pool,
                    # overlapping the DVE rescales and the next few widths
                    for g in range(G):
                        tCf = ap_of(consts, 4 * g * D + 26 * N,
                                    [[D, 4], [N, N - 26], [1, N]])
                        nc.gpsimd.tensor_tensor(
                            tCf, tCf,
                            ap_of(zb, ZB_MX + g * (w + 2) * N + N,
                                  [[0, 4], [0, N - 26], [1, N]]),
                            OP.mult,
                        )
                    for g in range(G):
                        mg = ZB_MX + g * (w + 2) * N
                        eng = v
                        # natural tables, rows d<=w: scale by 2^(-k*d)
                        tA = ap_of(banks, g * D, [[4 * D, 4], [N, w + 1], [1, N]])
                        eng.tensor_tensor(
                            tA, tA,
                            ap_of(zb, mg, [[0, 4], [N, w + 1], [1, N]]),
                            OP.mult,
                        )
                        # IR/IL rows r<=w-1 hold width r+1: scale 2^(-k*(r+1))
                        tI = ap_of(banks, (16 + g) * D, [[4 * D, 2], [N, w], [1, N]])
                        eng.tensor_tensor(
                            tI, tI,
                            ap_of(zb, mg + N, [[0, 2], [N, w], [1, N]]),
                            OP.mult,
                        )
                        # near const rows w+1..25: one extra arc factor 2^-k
                        tC = ap_of(consts, 4 * g * D + (w + 1) * N,
                                   [[D, 4], [N, 25 - w], [1, N]])
                        v.tensor_tensor(
                            tC, tC,
                            ap_of(zb, mg + N, [[0, 4], [0, 25 - w], [1, N]]),
                            OP.mult,
                        )

            # ---- extract raw exp-domain CR[0, j] (log on host) ----
            v.tensor_copy(
                ap_of(zf, ZF_CROUT, [[N, 4], [1, N]]),
                ap_of(banks, S_CRA * D, [[D, 4], [N, N]]),
            )
            nc.sync.dma_start(
                ap_of(logs_d, 0, [[N, G], [1, N]], lead=[G * N, 128]),
                ap_of(zf, ZF_CROUT, [[N, G], [1, N]]),
            )
            nc.sync.dma_start(
                ap_of(dsum_d, 0, [[1, G]], lead=[G, 128]),
                ap_of(zf, ZF_DSUM, [[1, G]]),
            )

    nc.compile()
    return nc


_NC_CACHE = {}


def get_nc():
    if "nc" not in _NC_CACHE:
        _NC_CACHE["nc"] = build_nc()
    return _NC_CACHE["nc"]


def make_in_maps(trans_scores, dec_scores):
    t = np.asarray(trans_scores, dtype=np.float32)
    dec = np.asarray(dec_scores, dtype=np.float32)
    B = t.shape[0]
    go = dec[..., 0]                        # [B, n, dir, dv]
    # per-sentence linear pre-shift: each arc factor carries exp(-c0), so a
    # width-w entry is scaled exp(-c0*w); undone on the host at the end.
    tm = np.where(t < -1e8, -np.inf, t).max(axis=3)
    with np.errstate(invalid="ignore"):
        colmax = tm.max(axis=1)             # [B, n] best arc into each child
        proxy = np.nanmean(
            np.where(np.isfinite(colmax), colmax, np.nan)[:, 1:], axis=-1)
    c0 = (proxy + 0.5).astype(np.float32)
    c0 = np.clip(np.nan_to_num(c0), -20.0, 20.0)
    # one exp over trans (NEG -> 0 underflow is intended), then gather diags
    with np.errstate(under="ignore"):
        E = np.exp(t - c0[:, None, None, None])      # [B, n, n, 2]
        ego = np.exp(go)                             # [B, n, 2, 2]
    d_idx, i_idx = np.meshgrid(np.arange(N), np.arange(N), indexing="ij")
    j_idx = np.minimum(i_idx + d_idx, N - 1)
    valid = ((i_idx + d_idx) <= N - 1)[None].astype(np.float32)
    ea = E[:, i_idx, j_idx, :]              # [B, n, n, 2]  trans[i, i+d, v]
    eb = E[:, j_idx, i_idx, :]              # [B, n, n, 2]  trans[i+d, i, v]
    a1 = ea[..., 1] * ego[:, :, 1, 1][:, i_idx] * valid
    a0 = ea[..., 0] * ego[:, :, 1, 0][:, i_idx] * valid
    b1 = eb[..., 1] * ego[:, :, 0, 1][:, j_idx] * valid
    b0 = eb[..., 0] * ego[:, :, 0, 0][:, j_idx] * valid
    consts = np.empty((B, 4, N, N), dtype=np.float32)
    consts[:, 0] = a1
    consts[:, 1] = b1
    consts[:, 2] = a0
    consts[:, 3] = b0
    consts = consts.reshape(B, CONST_IN).astype(ml_dtypes.bfloat16)
    est = np.exp(dec[..., 1])               # [B, n, dir, dv]
    stops = np.empty((B, 4, N), dtype=np.float32)
    stops[:, 0] = est[:, :, 0, 0]; stops[:, 1] = est[:, :, 0, 1]
    stops[:, 2] = est[:, :, 1, 0]; stops[:, 3] = est[:, :, 1, 1]
    stops = stops.reshape(B, STOP_IN).astype(ml_dtypes.bfloat16)
    in_maps = []
    for c in range(NCORES):
        sl = slice(c * B_CORE, (c + 1) * B_CORE)
        in_maps.append({
            "consts": consts[sl],
            "stops": stops[sl],
        })
    return in_maps, c0


L0_HOST = 5  # sentences with len <= L0_HOST are computed exactly on the host


def _host_short_ll(trans, dec, lens):
    """Exact f64 LL for short sentences via the inside DP truncated to
    positions 0..L0_HOST (spans of a length-l sentence live within [0, l])."""
    n = L0_HOST + 1
    t = np.asarray(trans)[:, :n, :n, :].astype(np.float64)
    dc = np.asarray(dec)[:, :n].astype(np.float64)
    B = t.shape[0]
    go = dc[..., 0]
    stop = dc[..., 1]
    NEG = -1e9
    IR = np.full((B, n, n), NEG)
    IL = np.full((B, n, n), NEG)
    KR = np.full((B, n, n), NEG)
    KL = np.full((B, n, n), NEG)
    dg = np.arange(n)
    KR[:, dg, dg] = 0.0
    KL[:, dg, dg] = 0.0
    CR = np.full((B, n, n), NEG)
    CL = np.full((B, n, n), NEG)
    CR[:, dg, dg] = stop[:, :, 1, 0]
    CL[:, dg, dg] = stop[:, :, 0, 0]
    goR = go[:, :, 1, :]
    goL = go[:, :, 0, :]

    def lse(x):
        m = x.max(axis=-1, keepdims=True)
        return np.squeeze(m, -1) + np.log(np.exp(x - m).sum(axis=-1))

    for w in range(1, n):
        s = n - w
        i = np.arange(s)[:, None]
        tt = np.arange(w)[None, :]
        j = i + w
        k = i + tt
        vR = (tt > 0).astype(np.int64)
        ir = lse(KR[:, i, k] + goR[:, i, vR] + t[:, i, j, np.minimum(vR, 1)]
                 + CL[:, k + 1, j])
        vL = (tt < w - 1).astype(np.int64)
        il = lse(CR[:, i, k] + KL[:, k + 1, j] + goL[:, j, vL]
                 + t[:, j, i, np.minimum(vL, 1)])
        i1 = np.arange(s)
        IR[:, i1, i1 + w] = ir
        IL[:, i1, i1 + w] = il
        kr = lse(IR[:, i, i + 1 + tt] + CR[:, i + 1 + tt, j])
        kl = lse(CL[:, i, i + tt] + IL[:, i + tt, j])
        KR[:, i1, i1 + w] = kr
        KL[:, i1, i1 + w] = kl
        CR[:, i1, i1 + w] = kr + stop[:, i1, 1, 1]
        CL[:, i1, i1 + w] = kl + stop[:, i1 + w, 0, 1]

    return CR[np.arange(B), 0, lens].astype(np.float32)


def assemble(results, len_array, c0):
    ln = np.asarray(len_array).astype(np.int64)
    c0 = np.asarray(c0).astype(np.float64)
    out = np.empty(len(ln), dtype=np.float32)
    for c, res in enumerate(results):
        ecr = res["ecr"].reshape(B_CORE, N).astype(np.float64)
        dsum = res["dsum"].reshape(B_CORE).astype(np.float64)
        lc = ln[c * B_CORE:(c + 1) * B_CORE]
        idx = np.arange(B_CORE)
        with np.errstate(divide="ignore"):
            out[c * B_CORE:(c + 1) * B_CORE] = (
                np.log(ecr[idx, lc]) + dsum * np.log(2.0) * lc
                + c0[c * B_CORE:(c + 1) * B_CORE] * lc
            ).astype(np.float32)
    return out


def kernel(trans_scores, dec_scores, len_array):
    from concourse.bass_utils import run_bass_kernel_spmd

    nc = get_nc()
    in_maps, c0 = make_in_maps(trans_scores, dec_scores)
    res = run_bass_kernel_spmd(nc, in_maps, core_ids=list(range(NCORES)))
    out = assemble(res.results, len_array, c0)
    lens = np.asarray(len_array).astype(np.int64)
    short = lens <= L0_HOST
    if short.any():
        out[short] = _host_short_ll(
            np.asarray(trans_scores)[short], np.asarray(dec_scores)[short],
            lens[short])
    return out


# revision 28
# speedup vs baseline: 1.0571x; 1.0182x over previous
"""DMV inside algorithm (Eisner chart DP, logsumexp semiring) on Trainium2.

Strategy
--------
Pure data parallelism over the batch: 4096 sentences -> 8 cores x 512.
Per core: ONE run of 512 sentences laid out as [128 SBUF partitions] x
[G=4 sentence groups in the free dim], all chart tables in bf16.

The DP runs in the *exp domain* (no per-split transcendentals): tables hold
exp(score). Each width-w update is one fused strided multiply (products
P[qg,t,i]) followed by an in-place folding tree of adds that reduces over
the split dim t. Everything iterates [qg, t, i] with i innermost and
stride 1, which (with bf16) hits the DVE 2x packed-16-bit mode; the tree
of tensor_tensor adds also runs at 2x, unlike InstTensorReduce which gets
no fast mode.

Tables are stored *diagonal-packed*: Xd[d*41 + i] = X[i, i+d], making
every gather in the width-w recurrence a constant-stride access pattern.
IR/IL are stored with row r holding width r+1 (IL column-shifted by +1)
so all four quantities' gathers share one AP shape.

Numerics: scale composes linearly in span width (every width-w entry has
exactly w arcs), so one on-device renormalization at w=20 multiplies row
d by an exact power of two 2^(-k*d) (k integer per sentence), keeping
everything in range (bf16 range == f32 range). k is returned per sentence
and undone on the host: LL = log(CR[0,len]) + k*ln2*len + c0*len.
"""

import os

os.environ.setdefault("JAX_PLATFORMS", "cpu")

import numpy as np
import ml_dtypes

import concourse.bass as bass  # noqa: F401  (registers engine classes)
import concourse.tile as tile
import bass_rust
from concourse import bacc, mybir

F32 = mybir.dt.float32
BF16 = mybir.dt.bfloat16
AF = mybir.ActivationFunctionType
OP = mybir.AluOpType
AX = mybir.AxisListType

N = 41              # fake_len (ROOT at 0)
D = 1681            # table pitch: N*N elements
G = 4               # sentence groups per partition
NCORES = 8
B_CORE = 128 * G    # 512
CONST_IN = 4 * D    # host sends 4 pre-exponentiated tables/sentence (bf16)
STOP_IN = 4 * N     # host sends 4 exp'd stop vectors/sentence (bf16)
RENORM_W = 20

# banks tile (bf16): 24 diag-packed tables, slot k at offset k*D.
# Since every DP multiply is a per-direction 4-slot op, no table needs a
# second copy; all paired-access strides stay within 16*D (ISA AP steps
# are 16-bit).
#   KL g0..g3: 0..3    KR: 4..7    CRa: 8..11   CLa: 12..15
#   IR: 16..19         IL: 20..23  (IL stored col+1)
S_KL, S_KR, S_CRA, S_CLA, S_IR, S_IL = 0, 4, 8, 12, 16, 20
# consts tile (bf16): 4 per-arc tables, g-major: offset (4*g + C)*D
# (A0/B0 kept verbatim, not as differences: all-positive arithmetic so
# bf16 never hits catastrophic cancellation)
C_A1, C_B1, C_A0, C_B0 = range(4)
# stops tile (bf16): 16 vectors of 41, offset (g*4 + v)*41
V_SLNO, V_SLHAS, V_SRNO, V_SRHAS = range(4)

# bf16 scratch tile element offsets
ZB_P = 0          # 3360: products [qg, t, i] (qg-stride = s*w, varies by w)
ZB_T1 = 3360      # 328: tmp1 [q, g, i]
ZB_T2 = 3688      # 328
ZB_SS = 4016      # 328: reduce-path sums [q, g, i]
ZB_MX = 4344      # 4*22*41: expanded renorm multiplier Mx[g, d, i] = 2^(-k_g*d)
ZB_TOTAL = 7952

# f32 scratch tile element offsets
ZF_M2 = 0         # 8
ZF_MU = 8         # 4
ZF_LM = 12        # 4 (reused for k)
ZF_M = 16         # 4*42: renorm multiplier table [g, 42]
ZF_CROUT = 184    # 4*41
ZF_DSUM = 348     # 4
ZF_TOTAL = 352

LN2_32 = 32.0 * float(np.log(2.0))

# from this width on, the g3 lane of opA-L / opB-R (mult + folds) runs on
# the otherwise-idle Pool engine; below it Pool's per-op launch overhead
# would make the Pool chain longer than the whole DVE op
POOL_W0 = 4
POOL2_LO, POOL2_HI = 99, 0  # widths with two Pool lanes (off by default)


def ap_of(t, offset, dims, lead=None):
    """Build a raw AP on tile/dram ap `t`: [lead or t.ap[0]] + dims."""
    ap = t.copy()
    first = list(t.ap[0]) if lead is None else list(lead)
    ap.ap = bass_rust.VecI64Pair([first] + [list(d) for d in dims])
    ap.offset = offset
    return ap


def build_nc():
    nc = bacc.Bacc("TRN2", target_bir_lowering=False, debug=False, num_devices=1)
    consts_in = nc.dram_tensor("consts", [B_CORE, CONST_IN], BF16, kind="ExternalInput").ap()
    stops_in = nc.dram_tensor("stops", [B_CORE, STOP_IN], BF16, kind="ExternalInput").ap()
    logs_d = nc.dram_tensor("ecr", [B_CORE, N], F32, kind="ExternalOutput").ap()
    dsum_d = nc.dram_tensor("dsum", [B_CORE], F32, kind="ExternalOutput").ap()

    with tile.TileContext(nc) as tc:
        with tc.tile_pool(name="p", bufs=1) as pool, \
                nc.allow_low_precision(reason="bf16 chart DP by design"):
            banks_t = pool.tile([128, 24 * D], BF16)
            consts_t = pool.tile([128, 16 * D], BF16)
            stops_t = pool.tile([128, 16 * N], BF16)
            zb_t = pool.tile([128, ZB_TOTAL], BF16)
            zf_t = pool.tile([128, ZF_TOTAL], F32)
            banks = banks_t[:]
            consts = consts_t[:]
            stops = stops_t[:]
            zb = zb_t[:]
            zf = zf_t[:]

            v = nc.vector
            sc = nc.scalar

            # ---- load host-precomputed exp-domain constants ----
            nc.sync.dma_start(
                ap_of(stops, 0, [[STOP_IN, G], [1, STOP_IN]]),
                ap_of(stops_in, 0,
                      [[STOP_IN, G], [1, STOP_IN]], lead=[G * STOP_IN, 128]),
            )
            # consts split by row range: step w reads row w only, so later
            # chunks' DMA hides under early DP steps
            # chunks of >=7 rows: a chunk of one table is (hi-lo)*82B
            # contiguous in DRAM, and >=512B descriptors avoid the small-
            # descriptor DMA penalty
            for lo, hi in ((0, 3 * N), (3 * N, 10 * N), (10 * N, 18 * N),
                           (18 * N, 26 * N), (26 * N, 34 * N), (34 * N, D)):
                nc.sync.dma_start(
                    ap_of(consts, lo, [[4 * D, G], [D, 4], [1, hi - lo]]),
                    ap_of(consts_in, lo, [[CONST_IN, G], [D, 4], [1, hi - lo]],
                          lead=[G * CONST_IN, 128]),
                )

            # ---- width-0 init ----
            # The renorm rescale reads full N-wide rows; the tail columns the
            # DP never writes hold whatever SBUF held before. That is safe:
            # those cells are only ever read by the renorm itself, which
            # multiplies them in place — garbage never reaches a DP-read cell,
            # so no zero-init is needed at all.
            v.memset(ap_of(zf, ZF_DSUM, [[1, G]]), 0.0)
            # KR[0,:] = KL[0,:] = 1
            v.memset(ap_of(banks, S_KL * D, [[4 * D, 2], [D, 4], [1, N]]), 1.0)
            # CRa[0,i] = exp(stop[i,R,NO]); CLa[0,i] = exp(stop[i,L,NO])
            v.tensor_copy(
                ap_of(banks, S_CRA * D, [[D, 4], [1, N]]),
                ap_of(stops, V_SRNO * N, [[4 * N, 4], [1, N]]),
            )
            v.tensor_copy(
                ap_of(banks, S_CLA * D, [[D, 4], [1, N]]),
                ap_of(stops, V_SLNO * N, [[4 * N, 4], [1, N]]),
            )

            def fold_chains(w, s, t0, count, chains):
                """In-place fold slot-chains of P[qg, t, i] over t in
                [t0, t0+count) down to one row at t0, interleaving the
                independent chains so they hide each other's semaphore
                latency. Each chain is (engine, first_slot, n_slots, final):
                `final` (if given) redirects that chain's last fold."""
                sw = s * w
                h = count
                while h > 1:
                    h2 = h // 2
                    hc = h - h2
                    for eng, slot0, nsl, fin in chains:
                        base = ZB_P + slot0 * sw + t0 * s
                        if hc == 1 and fin is not None:
                            out = fin
                        else:
                            out = ap_of(zb, base, [[sw, nsl], [s, h2], [1, s]])
                        eng.tensor_tensor(
                            out,
                            ap_of(zb, base, [[sw, nsl], [s, h2], [1, s]]),
                            ap_of(zb, base + hc * s, [[sw, nsl], [s, h2], [1, s]]),
                            OP.add,
                        )
                    h = hc

            # ---- chart DP ----
            for w in range(1, N):
                s = N - w
                sw = s * w
                row = (w - 1) * N + 1
                # opA: NOCHILD edges (t=0 for q=R, t=w-1 for q=L) have a
                # trivial K-factor of 1, so products cover only the w-1
                # HASCHILD splits; the edges are read straight from banks.
                # The mults depend only on last width's fold finals (not its
                # stop-mult), so they are emitted first; tmp2 (which reads the
                # stop-mult's rows) runs under their shadow.
                if w > 1:
                    sw1 = s * (w - 1)
                    npl = 0 if w < POOL_W0 else (2 if POOL2_LO <= w <= POOL2_HI else 1)
                    # q=R forward, t in [1,w): P[g,t,i] = KR[t,i]*CLa[w-1-t, i+t+1]
                    v.tensor_tensor(
                        ap_of(zb, ZB_P, [[sw1, 4], [s, w - 1], [1, s]]),
                        ap_of(banks, S_KR * D + N, [[D, 4], [N, w - 1], [1, s]]),
                        ap_of(banks, S_CLA * D + row - 40, [[D, 4], [-40, w - 1], [1, s]]),
                        OP.mult,
                    )
                    # q=L t-reversed, t' in [1,w): P[4+g,t',i] = CRa[w-1-t',i]*KL[t', i+w-t']
                    # (trailing g-lanes of this half run on the idle Pool engine)
                    ng = 4 - npl
                    v.tensor_tensor(
                        ap_of(zb, ZB_P + 4 * sw1, [[sw1, ng], [s, w - 1], [1, s]]),
                        ap_of(banks, S_CRA * D + (w - 2) * N, [[D, ng], [-N, w - 1], [1, s]]),
                        ap_of(banks, S_KL * D + N + w - 1, [[D, ng], [40, w - 1], [1, s]]),
                        OP.mult,
                    )
                    if npl:
                        nc.gpsimd.tensor_tensor(
                            ap_of(zb, ZB_P + (4 + ng) * sw1, [[sw1, npl], [s, w - 1], [1, s]]),
                            ap_of(banks, (S_CRA + ng) * D + (w - 2) * N, [[D, npl], [-N, w - 1], [1, s]]),
                            ap_of(banks, (S_KL + ng) * D + N + w - 1, [[D, npl], [40, w - 1], [1, s]]),
                            OP.mult,
                        )
                    # tmp2[q,g,i] = NOCHILD edge * {A0,B0}[w,:]
                    v.tensor_tensor(
                        ap_of(zb, ZB_T2, [[4 * s, 2], [s, 4], [1, s]]),
                        ap_of(banks, S_CLA * D + (w - 1) * N + 1,
                              [[-4 * D - 1, 2], [D, 4], [1, s]]),
                        ap_of(consts, C_A0 * D + w * N, [[D, 2], [4 * D, 4], [1, s]]),
                        OP.mult,
                    )
                    if npl:
                        fold_chains(w - 1, s, 0, w - 1, [
                            (v, 0, 4, None), (v, 4, ng, None), (nc.gpsimd, 4 + ng, npl, None)])
                    else:
                        fold_chains(w - 1, s, 0, w - 1, [(v, 0, 4, None), (v, 4, 4, None)])
                    # tmp1 halves and the IR/IL writes alternate between the
                    # R and L chains so each hides the other's sem latency
                    v.tensor_tensor(
                        ap_of(zb, ZB_T1, [[s, 4], [1, s]]),
                        ap_of(zb, ZB_P, [[sw1, 4], [1, s]]),
                        ap_of(consts, C_A1 * D + w * N, [[4 * D, 4], [1, s]]),
                        OP.mult,
                    )
                    v.tensor_tensor(
                        ap_of(zb, ZB_T1 + 4 * s, [[s, 4], [1, s]]),
                        ap_of(zb, ZB_P + 4 * sw1, [[sw1, 4], [1, s]]),
                        ap_of(consts, C_B1 * D + w * N, [[4 * D, 4], [1, s]]),
                        OP.mult,
                    )
                    # IR[w-1, i] = tmp1R + tmp2R; IL[w-1, i+1] = tmp1L + tmp2L
                    v.tensor_tensor(
                        ap_of(banks, S_IR * D + (w - 1) * N, [[D, 4], [1, s]]),
                        ap_of(zb, ZB_T1, [[s, 4], [1, s]]),
                        ap_of(zb, ZB_T2, [[s, 4], [1, s]]),
                        OP.add,
                    )
                    v.tensor_tensor(
                        ap_of(banks, S_IL * D + (w - 1) * N + 1, [[D, 4], [1, s]]),
                        ap_of(zb, ZB_T1 + 4 * s, [[s, 4], [1, s]]),
                        ap_of(zb, ZB_T2 + 4 * s, [[s, 4], [1, s]]),
                        OP.add,
                    )
                else:
                    # w=1: only the NOCHILD edge exists: IR/IL row 0 = tmp2
                    v.tensor_tensor(
                        ap_of(zb, ZB_T2, [[4 * s, 2], [s, 4], [1, s]]),
                        ap_of(banks, S_CLA * D + (w - 1) * N + 1,
                              [[-4 * D - 1, 2], [D, 4], [1, s]]),
                        ap_of(consts, C_A0 * D + w * N, [[D, 2], [4 * D, 4], [1, s]]),
                        OP.mult,
                    )
                    v.tensor_copy(
                        ap_of(banks, S_IR * D, [[4 * D + 1, 2], [D, 4], [1, s]]),
                        ap_of(zb, ZB_T2, [[4 * s, 2], [s, 4], [1, s]]),
                    )
                # opB products, half 0 (q=L): P[g,t,i] = CLa[t,i]*IL[w-1-t, i+t+1]
                # half 1 (q=R): P[4+g,t,i] = IR[t,i]*CRa[w-1-t, i+t+1]
                klout = ap_of(banks, S_KL * D + w * N, [[D, 4], [1, s]])
                krout = ap_of(banks, S_KR * D + w * N, [[D, 4], [1, s]])
                if w == 1:
                    v.tensor_tensor(
                        klout,
                        ap_of(banks, S_CLA * D, [[D, 4], [N, 1], [1, s]]),
                        ap_of(banks, S_IL * D + row, [[D, 4], [-40, 1], [1, s]]),
                        OP.mult,
                    )
                    v.tensor_tensor(
                        krout,
                        ap_of(banks, S_IR * D, [[D, 4], [N, 1], [1, s]]),
                        ap_of(banks, S_CRA * D + row, [[D, 4], [-40, 1], [1, s]]),
                        OP.mult,
                    )
                else:
                    npl = 0 if w < POOL_W0 else (2 if POOL2_LO <= w <= POOL2_HI else 1)
                    ng = 4 - npl
                    v.tensor_tensor(
                        ap_of(zb, ZB_P + 4 * sw, [[sw, ng], [s, w], [1, s]]),
                        ap_of(banks, S_IR * D, [[D, ng], [N, w], [1, s]]),
                        ap_of(banks, S_CRA * D + row, [[D, ng], [-40, w], [1, s]]),
                        OP.mult,
                    )
                    if npl:
                        nc.gpsimd.tensor_tensor(
                            ap_of(zb, ZB_P + (4 + ng) * sw, [[sw, npl], [s, w], [1, s]]),
                            ap_of(banks, (S_IR + ng) * D, [[D, npl], [N, w], [1, s]]),
                            ap_of(banks, (S_CRA + ng) * D + row, [[D, npl], [-40, w], [1, s]]),
                            OP.mult,
                        )
                    v.tensor_tensor(
                        ap_of(zb, ZB_P, [[sw, 4], [s, w], [1, s]]),
                        ap_of(banks, S_CLA * D, [[D, 4], [N, w], [1, s]]),
                        ap_of(banks, S_IL * D + row, [[D, 4], [-40, w], [1, s]]),
                        OP.mult,
                    )
                    if npl:
                        krp = ap_of(banks, (S_KR + ng) * D + w * N, [[D, npl], [1, s]])
                        krd = ap_of(banks, S_KR * D + w * N, [[D, ng], [1, s]])
                        fold_chains(w, s, 0, w, [
                            (v, 4, ng, krd), (nc.gpsimd, 4 + ng, npl, krp), (v, 0, 4, klout)])
                    else:
                        fold_chains(w, s, 0, w, [(v, 4, 4, krout), (v, 0, 4, klout)])
                # CRa[w,i] = KR[w,i]*sRhas[i]; CLa[w,i] = KL[w,i]*sLhas[i+w]
                v.tensor_tensor(
                    ap_of(banks, S_CRA * D + w * N, [[4 * D, 2], [D, 4], [1, s]]),
                    ap_of(banks, S_KR * D + w * N, [[-4 * D, 2], [D, 4], [1, s]]),
                    ap_of(stops, V_SRHAS * N, [[w - 2 * N, 2], [4 * N, 4], [1, s]]),
                    OP.mult,
                )

                if w == RENORM_W:
                    s0 = N - w
                    # mu[g] = max_i max(KR[w,i], KL[w,i])  (per partition)
                    v.tensor_reduce(
                        ap_of(zf, ZF_M2, [[4, 2], [1, 4]]),
                        ap_of(banks, S_KL * D + w * N, [[4 * D, 2], [D, 4], [1, s0]]),
                        axis=AX.X, op=OP.max,
                    )
                    v.tensor_tensor(
                        ap_of(zf, ZF_MU, [[1, 4]]),
                        ap_of(zf, ZF_M2, [[1, 4]]),
                        ap_of(zf, ZF_M2 + 4, [[1, 4]]),
                        OP.max,
                    )
                    # Ln range on ACT is +-2^64: compute via mu*2^-32
                    v.tensor_scalar_mul(
                        ap_of(zf, ZF_MU, [[1, 4]]), ap_of(zf, ZF_MU, [[1, 4]]), 2.0**-32
                    )
                    v.tensor_scalar_max(
                        ap_of(zf, ZF_MU, [[1, 4]]), ap_of(zf, ZF_MU, [[1, 4]]), 1e-36
                    )
                    sc.activation(
                        ap_of(zf, ZF_LM, [[1, 4]]), ap_of(zf, ZF_MU, [[1, 4]]), AF.Ln
                    )
                    # quantize the per-width shift to delta = -k*ln2 with k
                    # integer, so every rescale factor is an EXACT power of
                    # two (exact in bf16 as well).
                    # kf = round((log(mu*2^-32) + 32 ln2) / (w ln2))
                    v.tensor_scalar(
                        ap_of(zf, ZF_LM, [[1, 4]]), ap_of(zf, ZF_LM, [[1, 4]]),
                        LN2_32, 1.0 / (w * float(np.log(2.0))),
                        OP.add, OP.mult,
                    )
                    v.tensor_scalar(
                        ap_of(zf, ZF_LM, [[1, 4]]), ap_of(zf, ZF_LM, [[1, 4]]),
                        12582912.0, 12582912.0, OP.add, OP.subtract,
                    )
                    # dsum accumulates k (exact small integers)
                    v.tensor_tensor(
                        ap_of(zf, ZF_DSUM, [[1, 4]]),
                        ap_of(zf, ZF_DSUM, [[1, 4]]),
                        ap_of(zf, ZF_LM, [[1, 4]]),
                        OP.add,
                    )
                    # scale2 = 2^-k via exponent bits: (127 - k) << 23
                    v.tensor_scalar(
                        ap_of(zf, ZF_M2, [[1, 4]]), ap_of(zf, ZF_LM, [[1, 4]]),
                        -1.0, 127.0, OP.mult, OP.add,
                    )
                    zi = zf.bitcast(mybir.dt.int32)
                    v.tensor_copy(
                        ap_of(zi, ZF_M2 + 4, [[1, 4]]),
                        ap_of(zf, ZF_M2, [[1, 4]]),
                    )
                    v.tensor_scalar(
                        ap_of(zi, ZF_M2 + 4, [[1, 4]]),
                        ap_of(zi, ZF_M2 + 4, [[1, 4]]),
                        23, None, OP.arith_shift_left,
                    )
                    # M[g, d] = 2^(-k*d): d=0 -> 1, then multiplicative scan
                    v.memset(ap_of(zf, ZF_M, [[42, 4], [1, 1]]), 1.0)
                    for g in range(G):
                        sca = ap_of(zf, ZF_M2 + 4 + g, [[0, 41]])
                        v.tensor_tensor_scan(
                            ap_of(zf, ZF_M + g * 42 + 1, [[1, 41]]),
                            sca, sca, 1.0, OP.mult, OP.bypass,
                        )
                    # expand to Mx[g, d, i] = M[g, d] (packed bf16, exact
                    # powers of two) so the rescales hit the 2x DVE mode
                    v.tensor_copy(
                        ap_of(zb, ZB_MX, [[(w + 2) * N, 4], [N, w + 2], [1, N]]),
                        ap_of(zf, ZF_M, [[42, 4], [1, w + 2], [0, N]]),
                    )
                    # far const rows (needed from w=26 on) rescale on Pool,
                    # overlapping the DVE rescales and the next few widths
                    for g in range(G):
                        tCf = ap_of(consts, 4 * g * D + 26 * N,
                                    [[D, 4], [N, N - 26], [1, N]])
                        nc.gpsimd.tensor_tensor(
                            tCf, tCf,
                            ap_of(zb, ZB_MX + g * (w + 2) * N + N,
                                  [[0, 4], [0, N - 26], [1, N]]),
                            OP.mult,
                        )
                    for g in range(G):
                        mg = ZB_MX + g * (w + 2) * N
                        eng = v
                        # natural tables, rows d<=w: scale by 2^(-k*d)
                        tA = ap_of(banks, g * D, [[4 * D, 4], [N, w + 1], [1, N]])
                        eng.tensor_tensor(
                            tA, tA,
                            ap_of(zb, mg, [[0, 4], [N, w + 1], [1, N]]),
                            OP.mult,
                        )
                        # IR/IL rows r<=w-1 hold width r+1: scale 2^(-k*(r+1))
                        tI = ap_of(banks, (16 + g) * D, [[4 * D, 2], [N, w], [1, N]])
                        eng.tensor_tensor(
                            tI, tI,
                            ap_of(zb, mg + N, [[0, 2], [N, w], [1, N]]),
                            OP.mult,
                        )
                        # near const rows w+1..25: one extra arc factor 2^-k
                        tC = ap_of(consts, 4 * g * D + (w + 1) * N,
                                   [[D, 4], [N, 25 - w], [1, N]])
                        v.tensor_tensor(
                            tC, tC,
                            ap_of(zb, mg + N, [[0, 4], [0, 25 - w], [1, N]]),
                            OP.mult,
                        )

            # ---- extract raw exp-domain CR[0, j] (log on host) ----
            v.tensor_copy(
                ap_of(zf, ZF_CROUT, [[N, 4], [1, N]]),
                ap_of(banks, S_CRA * D, [[D, 4], [N, N]]),
            )
            nc.sync.dma_start(
                ap_of(logs_d, 0, [[N, G], [1, N]], lead=[G * N, 128]),
                ap_of(zf, ZF_CROUT, [[N, G], [1, N]]),
            )
            nc.sync.dma_start(
                ap_of(dsum_d, 0, [[1, G]], lead=[G, 128]),
                ap_of(zf, ZF_DSUM, [[1, G]]),
            )

    nc.compile()
    return nc


_NC_CACHE = {}


def get_nc():
    if "nc" not in _NC_CACHE:
        _NC_CACHE["nc"] = build_nc()
    return _NC_CACHE["nc"]


def make_in_maps(trans_scores, dec_scores):
    t = np.asarray(trans_scores, dtype=np.float32)
    dec = np.asarray(dec_scores, dtype=np.float32)
    B = t.shape[0]
    go = dec[..., 0]                        # [B, n, dir, dv]
    # per-sentence linear pre-shift: each arc factor carries exp(-c0), so a
    # width-w entry is scaled exp(-c0*w); undone on the host at the end.
    tm = np.where(t < -1e8, -np.inf, t).max(axis=3)
    with np.errstate(invalid="ignore"):
        colmax = tm.max(axis=1)             # [B, n] best arc into each child
        proxy = np.nanmean(
            np.where(np.isfinite(colmax), colmax, np.nan)[:, 1:], axis=-1)
    c0 = (proxy + 0.5).astype(np.float32)
    c0 = np.clip(np.nan_to_num(c0), -20.0, 20.0)
    # one exp over trans (NEG -> 0 underflow is intended), then gather diags
    with np.errstate(under="ignore"):
        E = np.exp(t - c0[:, None, None, None])      # [B, n, n, 2]
        ego = np.exp(go)                             # [B, n, 2, 2]
    d_idx, i_idx = np.meshgrid(np.arange(N), np.arange(N), indexing="ij")
    j_idx = np.minimum(i_idx + d_idx, N - 1)
    valid = ((i_idx + d_idx) <= N - 1)[None].astype(np.float32)
    ea = E[:, i_idx, j_idx, :]              # [B, n, n, 2]  trans[i, i+d, v]
    eb = E[:, j_idx, i_idx, :]              # [B, n, n, 2]  trans[i+d, i, v]
    a1 = ea[..., 1] * ego[:, :, 1, 1][:, i_idx] * valid
    a0 = ea[..., 0] * ego[:, :, 1, 0][:, i_idx] * valid
    b1 = eb[..., 1] * ego[:, :, 0, 1][:, j_idx] * valid
    b0 = eb[..., 0] * ego[:, :, 0, 0][:, j_idx] * valid
    consts = np.empty((B, 4, N, N), dtype=np.float32)
    consts[:, 0] = a1
    consts[:, 1] = b1
    consts[:, 2] = a0
    consts[:, 3] = b0
    consts = consts.reshape(B, CONST_IN).astype(ml_dtypes.bfloat16)
    est = np.exp(dec[..., 1])               # [B, n, dir, dv]
    stops = np.empty((B, 4, N), dtype=np.float32)
    stops[:, 0] = est[:, :, 0, 0]; stops[:, 1] = est[:, :, 0, 1]
    stops[:, 2] = est[:, :, 1, 0]; stops[:, 3] = est[:, :, 1, 1]
    stops = stops.reshape(B, STOP_IN).astype(ml_dtypes.bfloat16)
    in_maps = []
    for c in range(NCORES):
        sl = slice(c * B_CORE, (c + 1) * B_CORE)
        in_maps.append({
            "consts": consts[sl],
            "stops": stops[sl],
        })
    return in_maps, c0


L0_HOST = 5  # sentences with len <= L0_HOST are computed exactly on the host


def _host_short_ll(trans, dec, lens):
    """Exact f64 LL for short sentences via the inside DP truncated to
    positions 0..L0_HOST (spans of a length-l sentence live within [0, l])."""
    n = L0_HOST + 1
    t = np.asarray(trans)[:, :n, :n, :].astype(np.float64)
    dc = np.asarray(dec)[:, :n].astype(np.float64)
    B = t.shape[0]
    go = dc[..., 0]
    stop = dc[..., 1]
    NEG = -1e9
    IR = np.full((B, n, n), NEG)
    IL = np.full((B, n, n), NEG)
    KR = np.full((B, n, n), NEG)
    KL = np.full((B, n, n), NEG)
    dg = np.arange(n)
    KR[:, dg, dg] = 0.0
    KL[:, dg, dg] = 0.0
    CR = np.full((B, n, n), NEG)
    CL = np.full((B, n, n), NEG)
    CR[:, dg, dg] = stop[:, :, 1, 0]
    CL[:, dg, dg] = stop[:, :, 0, 0]
    goR = go[:, :, 1, :]
    goL = go[:, :, 0, :]

    def lse(x):
        m = x.max(axis=-1, keepdims=True)
        return np.squeeze(m, -1) + np.log(np.exp(x - m).sum(axis=-1))

    for w in range(1, n):
        s = n - w
        i = np.arange(s)[:, None]
        tt = np.arange(w)[None, :]
        j = i + w
        k = i + tt
        vR = (tt > 0).astype(np.int64)
        ir = lse(KR[:, i, k] + goR[:, i, vR] + t[:, i, j, np.minimum(vR, 1)]
                 + CL[:, k + 1, j])
        vL = (tt < w - 1).astype(np.int64)
        il = lse(CR[:, i, k] + KL[:, k + 1, j] + goL[:, j, vL]
                 + t[:, j, i, np.minimum(vL, 1)])
        i1 = np.arange(s)
        IR[:, i1, i1 + w] = ir
        IL[:, i1, i1 + w] = il
        kr = lse(IR[:, i, i + 1 + tt] + CR[:, i + 1 + tt, j])
        kl = lse(CL[:, i, i + tt] + IL[:, i + tt, j])
        KR[:, i1, i1 + w] = kr
        KL[:, i1, i1 + w] = kl
        CR[:, i1, i1 + w] = kr + stop[:, i1, 1, 1]
        CL[:, i1, i1 + w] = kl + stop[:, i1 + w, 0, 1]

    return CR[np.arange(B), 0, lens].astype(np.float32)


def assemble(results, len_array, c0):
    ln = np.asarray(len_array).astype(np.int64)
    c0 = np.asarray(c0).astype(np.float64)
    out = np.empty(len(ln), dtype=np.float32)
    for c, res in enumerate(results):
        ecr = res["ecr"].reshape(B_CORE, N).astype(np.float64)
        dsum = res["dsum"].reshape(B_CORE).astype(np.float64)
        lc = ln[c * B_CORE:(c + 1) * B_CORE]
        idx = np.arange(B_CORE)
        with np.errstate(divide="ignore"):
            out[c * B_CORE:(c + 1) * B_CORE] = (
                np.log(ecr[idx, lc]) + dsum * np.log(2.0) * lc
                + c0[c * B_CORE:(c + 1) * B_CORE] * lc
            ).astype(np.float32)
    return out


def kernel(trans_scores, dec_scores, len_array):
    from concourse.bass_utils import run_bass_kernel_spmd

    nc = get_nc()
    in_maps, c0 = make_in_maps(trans_scores, dec_scores)
    res = run_bass_kernel_spmd(nc, in_maps, core_ids=list(range(NCORES)))
    out = assemble(res.results, len_array, c0)
    lens = np.asarray(len_array).astype(np.int64)
    short = lens <= L0_HOST
    if short.any():
        out[short] = _host_short_ll(
            np.asarray(trans_scores)[short], np.asarray(dec_scores)[short],
            lens[short])
    return out
